# revision 1
# baseline (speedup 1.0000x reference)
"""Trainium2 Bass kernel for nn_Attention: fused QKV + RoPE + softmax attention + o_proj.

Sharding (8 cores): core c -> (batch b = c//2, head-half g = c%2).
Each core computes 8 of 16 heads for one batch:
  - QKV projection (fp16 matmuls, weights pre-transposed/tiled on host)
  - RoPE on DVE (d-on-partition layout, rotation = partition-block swap)
  - scores^T [kpos, q] per head pair, row-packed via tile_position
  - exp on ACT (1/sqrt(d) folded into the activation scale), fp16 out
  - P@V with ones-augmented V (M=65): row 64 = softmax denominator, free
  - normalize+cast on DVE, o_proj with w_o columns sharded by head
  - partial out^T [H, S] fp32; host sums the two head-half partials per batch
Emission is globally software-pipelined: scores lead exp/P@V by 2; o_proj of
chunk qc-1 and Q-proj of qc+1 hide under the exp stream; head-phase K-proj
and V-proj interleave so the ACT and DVE copy streams overlap.
"""
import os
import sys

sys.path.insert(0, "/opt/trn_rl_repo")

_PHASE = os.environ.get("ATTN_PHASE", "all")  # timing bisection knob

import numpy as np
import ml_dtypes

import concourse.bass as bass
import concourse.mybir as mybir
import concourse.tile as tile
from concourse import library_config
from concourse.bass_utils import run_bass_kernel_spmd
from concourse.vector_clock import ScopedClock, VectorClock

# ---------------------------------------------------------------------------
# Patch TileContext._drain_and_barrier: the walrus build in this container
# allows only ONE sync-wait per instruction; Tile's tail drain carries one
# wait per active proc.  Split them into single-wait NOPs on SP.
N_PROCS = 27


def _patched_drain_and_barrier(self, tick_clock, wait_clock):
    nc = self.nc
    gc = tick_clock.global_clock
    for p in range(N_PROCS):
        t = gc[p]
        if t > 0:
            nop = nc.sync.nop(nofuse=True)
            vc = VectorClock([t if q == p else 0 for q in range(N_PROCS)])
            wait_clock.add_sem_waits(nop.ins, ScopedClock({None: vc}))
    nc.sync.drain()
    nc.all_engine_barrier()
    assert self.sems is not None
    popped = nc._tile_sem_poison_stack.pop()
    assert popped is self._sem_poison
    nc.clear_and_free_semaphores(list(self.sems.allocated().values()))
    nc.all_engine_barrier()


tile.TileContext._drain_and_barrier = _patched_drain_and_barrier


def _split_excess_waits(nc):
    """walrus in this container accepts 1 sync-wait per instruction (2 on
    EventSemaphore).  Move excess waits onto EventSemaphore instructions
    inserted just before, on the same engine."""
    for f in nc.m.functions:
        for bb in f.blocks:
            new_insts = []
            changed = False
            for ins in bb.instructions:
                si = ins.sync_info
                waits = list(si.on_wait) if si is not None else []
                cap = 2 if isinstance(ins, mybir.InstEventSemaphore) else 1
                if len(waits) > cap:
                    changed = True
                    excess = waits[: len(waits) - cap]
                    for i in range(0, len(excess), 2):
                        ev = mybir.InstEventSemaphore(
                            name=f"I-{nc.next_id()}",
                            engine=ins.engine,
                            ins=[],
                            outs=[],
                            sync_info=mybir.SyncInfo(
                                on_wait=excess[i : i + 2], on_update=[]
                            ),
                        )
                        nc.register_instruction(ev)
                        new_insts.append(ev)
                    si.on_wait = waits[len(waits) - cap :]
                new_insts.append(ins)
            if changed:
                bb.instructions[:] = new_insts
# ---------------------------------------------------------------------------

B, S, H, NH, HD = 4, 2048, 1024, 16, 64
HPC = NH // 2          # heads per core
PAIRS = HPC // 2       # head pairs per core
HT = H // 128          # hidden-dim tiles
QKF = 2 * HPC * HD     # q+k features per core (1024)
VF = HPC * HD          # v features per core (512)
SC = 512               # seq chunk (psum bank)
NSC = S // SC
KT = S // 128          # kpos tiles
BF = mybir.dt.float16  # fp16: 10-bit mantissa, same PE/DVE speed as bf16
F32 = mybir.dt.float32
EXP_SCALE = 1.0 / float(np.sqrt(HD))

_CACHED_NC = None


def _build_nc():
    nc = bass.Bass()
    hT = nc.declare_dram_parameter("hT", [128, HT, S], BF, isOutput=False)
    wqk = nc.declare_dram_parameter("wqk", [128, HT, QKF], BF, isOutput=False)
    wv = nc.declare_dram_parameter("wv", [128, HT, VF], BF, isOutput=False)
    wo = nc.declare_dram_parameter("wo", [128, VF // 128, H], BF, isOutput=False)
    cos_t = nc.declare_dram_parameter("cos_t", [128, S], BF, isOutput=False)
    sin_t = nc.declare_dram_parameter("sin_t", [128, S], BF, isOutput=False)
    outT = nc.declare_dram_parameter("outT", [H, S], F32, isOutput=True)

    Exp = mybir.ActivationFunctionType.Exp

    with tile.TileContext(nc) as tc:
        with tc.tile_pool(name="singles", bufs=1) as singles:
            hT_sb = singles.tile([128, HT, S], BF)
            wqk_sb = singles.tile([128, HT, QKF], BF)
            wv_sb = singles.tile([128, HT, VF], BF)
            wo_sb = singles.tile([128, VF // 128, H], BF)
            cos_sb = singles.tile([128, S], BF)
            sin_sb = singles.tile([128, S], BF)
            q_rope = singles.tile([128, PAIRS, S], BF)
            k_rope = singles.tile([128, PAIRS, S], BF)
            vext = singles.tile([128, KT, HPC * 65], BF)
            ones_sb = singles.tile([1, 64], F32)
            nc.vector.memset(ones_sb[:], 1.0)

            for k in range(HT):
                nc.sync.dma_start(out=hT_sb[:, k, :], in_=hT[:, k, :])
                nc.sync.dma_start(out=wqk_sb[:, k, :], in_=wqk[:, k, :])
                nc.sync.dma_start(out=wv_sb[:, k, :], in_=wv[:, k, :])
            nc.sync.dma_start(out=wo_sb[:], in_=wo[:])
            nc.sync.dma_start(out=cos_sb[:], in_=cos_t[:])
            nc.sync.dma_start(out=sin_sb[:], in_=sin_t[:])
            nc.gpsimd.memset(vext[:], 1.0)

            # ---- pools (PSUM: sps 4 + atps 2 + projps 1 + opps 1 = 8) ----
            with (
                tc.tile_pool(name="sps", bufs=2, space="PSUM") as sps,
                tc.tile_pool(name="atps", bufs=1, space="PSUM") as atps,
                tc.tile_pool(name="projps", bufs=1, space="PSUM") as projps,
                tc.tile_pool(name="opps", bufs=1, space="PSUM") as opps,
                tc.tile_pool(name="raws", bufs=3) as raws,
                tc.tile_pool(name="ropet", bufs=3) as ropet,
                tc.tile_pool(name="eps", bufs=8) as eps,
                tc.tile_pool(name="rps", bufs=4) as rps,
                tc.tile_pool(name="rbs", bufs=4) as rbs,
                tc.tile_pool(name="attns", bufs=2) as attns,
                tc.tile_pool(name="obs", bufs=4) as obs,
            ):

                def proj_chunk(m, c, in_stream=False):
                    """Project q/k feature tile m for seq chunk c, apply RoPE.

                    in_stream: allocate psum from the 1-bank projps pool so
                    the scores ping-pong slots stay free."""
                    pair = m % PAIRS
                    dst_t = q_rope if m < PAIRS else k_rope
                    if in_stream:
                        ps = projps.tile([128, SC], F32, tag="pj", name="pj")
                        ps = ps[:]
                    else:
                        ps2 = sps.tile([128, 2, SC], F32, tag="s2")
                        ps = ps2[:, 0, :]
                    for k in range(HT):
                        nc.tensor.matmul(
                            ps,
                            wqk_sb[:, k, m * 128 : (m + 1) * 128],
                            hT_sb[:, k, c * SC : (c + 1) * SC],
                            start=(k == 0),
                            stop=(k == HT - 1),
                        )
                    raw = raws.tile([128, SC], BF)
                    if in_stream:
                        # ACT is saturated with exp during the attention
                        # stream; DVE has slack there
                        nc.vector.tensor_copy(raw[:], ps)
                    else:
                        nc.scalar.copy(raw[:], ps)
                    cs = cos_sb[:, c * SC : (c + 1) * SC]
                    sn = sin_sb[:, c * SC : (c + 1) * SC]
                    dst = dst_t[:, pair, c * SC : (c + 1) * SC]
                    t1 = ropet.tile([128, SC], BF, tag="t1")
                    t2 = ropet.tile([128, SC], BF, tag="t2")
                    nc.vector.tensor_mul(t1[:], raw[:], cs)
                    nc.vector.tensor_mul(t2[0:32], raw[32:64], sn[32:64])
                    nc.vector.tensor_mul(t2[32:64], raw[0:32], sn[0:32])
                    nc.vector.tensor_mul(t2[64:96], raw[96:128], sn[96:128])
                    nc.vector.tensor_mul(t2[96:128], raw[64:96], sn[64:96])
                    nc.vector.tensor_add(dst, t1[:], t2[:])

                def v_proj(st):
                    ps2 = sps.tile([128, 2, SC], F32, tag="s2", name="vps")
                    ps = ps2[:, 0, :]
                    for k in range(HT):
                        nc.tensor.matmul(
                            ps,
                            hT_sb[:, k, st * 128 : (st + 1) * 128],
                            wv_sb[:, k, :],
                            start=(k == 0),
                            stop=(k == HT - 1),
                        )
                    vdst = vext[:, st, :].rearrange("p (h x) -> p h x", x=65)[:, :, 0:64]
                    vsrc = ps.rearrange("p (h x) -> p h x", x=64)
                    nc.vector.tensor_copy(vdst, vsrc)

                # Head phase: K projection (scores need all kpos) interleaved
                # with V projection so the ACT (K copies) and DVE (V copies,
                # RoPE) streams overlap instead of running phase-serially.
                for m in range(PAIRS, 2 * PAIRS):
                    for c in range(NSC):
                        proj_chunk(m, c)
                        v_proj((m - PAIRS) * NSC + c)

                # Q-proj for qc=0 (later q-chunks are projected in-stream)
                for m in range(PAIRS):
                    proj_chunk(m, 0)

                if _PHASE == "proj":
                    for c in range(1, NSC):
                        for m in range(PAIRS):
                            proj_chunk(m, c)
                    return nc

                def emit_scores(pair, kt, qsl):
                    ksl = slice(kt * 128, (kt + 1) * 128)
                    s2 = sps.tile([128, 2, SC], F32, tag="s2", name="s2")
                    nc.tensor.matmul(
                        s2[:, 0, :],
                        k_rope[0:64, pair, ksl],
                        q_rope[0:64, pair, qsl],
                        start=True,
                        stop=True,
                        tile_position=(0, 0),
                    )
                    nc.tensor.matmul(
                        s2[:, 1, :],
                        k_rope[64:128, pair, ksl],
                        q_rope[64:128, pair, qsl],
                        start=True,
                        stop=True,
                        tile_position=(64, 0),
                    )
                    return s2

                def o_proj_m(qc, m, attnq, alt_pool=False):
                    qsl = slice(qc * SC, (qc + 1) * SC)
                    if alt_pool:
                        # tail-only: ping-pong with the (then idle) projps
                        # bank so MM groups overlap the psum->sbuf copies
                        op = projps.tile([128, SC], F32, tag="pj", name="pj")
                    else:
                        op = opps.tile([128, SC], F32, tag="op", name="op")
                    for ot in range(VF // 128):
                        nc.tensor.matmul(
                            op[:],
                            wo_sb[:, ot, m * 128 : (m + 1) * 128],
                            attnq[:, ot, :],
                            start=(ot == 0),
                            stop=(ot == VF // 128 - 1),
                        )
                    ob = obs.tile([128, SC], F32, tag="ob", name="ob")
                    nc.vector.tensor_copy(ob[:], op[:])
                    nc.sync.dma_start(out=outT[m * 128 : (m + 1) * 128, qsl], in_=ob[:])

                # globally software-pipelined attention stream:
                #  - scores lead exp/P@V by 2 iterations
                #  - o_proj of qc-1 interleaved into pair 0 of qc
                #  - Q-proj of qc+1 interleaved into pair 3 of qc
                flat = [
                    (qc, p, k)
                    for qc in range(NSC)
                    for p in range(PAIRS)
                    for k in range(KT)
                ]

                def qsl_of(qc):
                    return slice(qc * SC, (qc + 1) * SC)

                s2q = {
                    0: emit_scores(flat[0][1], flat[0][2], qsl_of(flat[0][0])),
                    1: emit_scores(flat[1][1], flat[1][2], qsl_of(flat[1][0])),
                }
                ats = {}
                attn_tiles = {}
                for i, (qc, pair, kt) in enumerate(flat):
                    qsl = qsl_of(qc)
                    if pair == 0 and kt == 0:
                        attn_tiles[qc] = attns.tile(
                            [128, PAIRS, SC], BF, tag="attn", name="attn"
                        )
                    if kt == 0:
                        ats[pair] = (
                            atps.tile([65, SC], F32, tag="ata", name="ata"),
                            atps.tile([65, SC], F32, tag="atb", name="atb"),
                        )
                    s2 = s2q.pop(i)
                    e = eps.tile([128, 2, SC], BF)
                    nc.scalar.activation(out=e[:], in_=s2[:], func=Exp, scale=EXP_SCALE)
                    if i + 2 < len(flat):
                        nqc, npair, nkt = flat[i + 2]
                        s2q[i + 2] = emit_scores(npair, nkt, qsl_of(nqc))
                    ata, atb = ats[pair]
                    vx = vext[:, kt, :].rearrange("p (h x) -> p h x", x=65)
                    nc.tensor.matmul(
                        ata[:],
                        vx[:, 2 * pair, :],
                        e[:, 0, :],
                        start=(kt == 0),
                        stop=(kt == KT - 1),
                    )
                    nc.tensor.matmul(
                        atb[:],
                        vx[:, 2 * pair + 1, :],
                        e[:, 1, :],
                        start=(kt == 0),
                        stop=(kt == KT - 1),
                    )
                    # interleaved next-chunk Q projection (PE+DVE work that
                    # hides under the ACT exp stream)
                    if qc + 1 < NSC and pair >= 2 and kt % 8 == 4:
                        proj_chunk((pair - 2) * 2 + kt // 8, qc + 1, in_stream=True)
                    # interleaved previous-chunk output projection (one m-tile
                    # every other kt, spread over pairs 0-1)
                    if _PHASE != "attn" and qc > 0 and pair < 2 and kt % 4 == 2:
                        o_proj_m(
                            qc - 1,
                            pair * 4 + kt // 4,
                            attn_tiles[qc - 1],
                            alt_pool=(kt // 4) % 2 == 1,
                        )
                    if kt == KT - 1:
                        for half, at in ((0, ata), (1, atb)):
                            rec = rps.tile([1, SC], F32)
                            nc.vector.reciprocal(rec[:], at[64:65, :])
                            rpp = projps.tile([128, SC], F32, tag="pj", name="pj")
                            nc.tensor.matmul(
                                rpp[0:64, :], ones_sb[:], rec[:], start=True, stop=True
                            )
                            rb = rbs.tile([64, SC], F32)
                            nc.vector.tensor_copy(rb[:], rpp[0:64, :])
                            nc.vector.tensor_mul(
                                attn_tiles[qc][half * 64 : (half + 1) * 64, pair, :],
                                at[0:64, :],
                                rb[:],
                            )
                if _PHASE != "attn":
                    for m in range(HT):
                        o_proj_m(NSC - 1, m, attn_tiles[NSC - 1], alt_pool=(m % 2 == 1))
    _split_excess_waits(nc)
    return nc


def _prep_inputs(cos, sin, hidden_states, w_qkv, w_o):
    """Per-core host-side sharding/transpose/cast. Returns list of in_maps."""
    bf = np.float16
    cos = np.asarray(cos, np.float32)
    sin = np.asarray(sin, np.float32)
    hidden_states = np.asarray(hidden_states, np.float32)
    w_qkv = np.asarray(w_qkv, np.float32)
    w_o = np.asarray(w_o, np.float32)

    cosT = cos.T  # [64, S]
    cos_t = np.ascontiguousarray(np.tile(cosT, (2, 1))).astype(bf)
    # sin multiplier aligned to the *source* partitions of the rot ops:
    # rows [0:32] = +sin[32:64] (multiplies src q[0:32] -> dest [32:64]),
    # rows [32:64] = -sin[0:32] (multiplies src q[32:64] -> dest [0:32]).
    sinT = sin.T
    sin_t = np.ascontiguousarray(
        np.tile(np.concatenate([sinT[32:], -sinT[:32]], 0), (2, 1))
    ).astype(bf)

    in_maps = []
    for core in range(8):
        b, g = core // 2, core % 2
        hT = hidden_states[b].T  # [H, S]
        hT_t = np.ascontiguousarray(
            hT.reshape(HT, 128, S).transpose(1, 0, 2)
        ).astype(bf)
        qs, ks, vs = g * VF, NH * HD + g * VF, 2 * NH * HD + g * VF
        wqk_rows = np.concatenate(
            [w_qkv[qs : qs + VF], w_qkv[ks : ks + VF]], 0
        )  # [QKF, H]
        wqk_t = np.ascontiguousarray(
            wqk_rows.T.reshape(HT, 128, QKF).transpose(1, 0, 2)
        ).astype(bf)
        wv_t = np.ascontiguousarray(
            w_qkv[vs : vs + VF].T.reshape(HT, 128, VF).transpose(1, 0, 2)
        ).astype(bf)
        woT = w_o[:, g * VF : (g + 1) * VF].T  # [VF, H]
        wo_t = np.ascontiguousarray(
            woT.reshape(VF // 128, 128, H).transpose(1, 0, 2)
        ).astype(bf)
        in_maps.append(
            {
                "hT": hT_t,
                "wqk": wqk_t,
                "wv": wv_t,
                "wo": wo_t,
                "cos_t": cos_t,
                "sin_t": sin_t,
            }
        )
    return in_maps


def kernel(cos, sin, hidden_states, w_qkv, w_o, _trace=False):
    global _CACHED_NC
    if _CACHED_NC is None:
        _CACHED_NC = _build_nc()
    nc = _CACHED_NC
    in_maps = _prep_inputs(cos, sin, hidden_states, w_qkv, w_o)
    res = run_bass_kernel_spmd(nc, in_maps, core_ids=list(range(8)), trace=_trace)
    outs = [r["outT"] for r in res.results]
    out = np.empty((B, S, H), np.float32)
    for b in range(B):
        out[b] = (outs[2 * b] + outs[2 * b + 1]).T
    if _trace:
        return out, res
    return out



# revision 25
# speedup vs baseline: 1.2304x; 1.2304x over previous
"""Trainium2 Bass kernel for nn_Attention: fused QKV + RoPE + softmax attention + o_proj.

Sharding (8 cores): core c -> (batch b = c//2, head-half g = c%2).
Each core computes 8 of 16 heads for one batch:
  - QKV projection (fp16 matmuls, weights pre-transposed/tiled on host)
  - RoPE on DVE (d-on-partition layout, rotation = partition-block swap)
  - scores^T [kpos, q] per head pair, row-packed via tile_position
  - exp on ACT (1/sqrt(d) folded into the activation scale), fp16 out
  - P@V transposed: probs chunk is the *stationary* operand, V (64 dims +
    ones column = 65) is the moving operand -> out [128 q, 65] accumulated
    over kpos; column 64 is the softmax denominator
  - normalize on DVE with a per-partition reciprocal scalar (denominator
    now lives on the q partition), fp16 attn [q, d]
  - attn [q, d] -> [d, q] via DMA XBAR transpose (SBUF->SBUF, no PE/PSUM)
  - o_proj with w_o columns sharded by head; partial out^T [H, S] fp32;
    host sums the two head-half partials per batch
Emission is globally software-pipelined: scores lead exp/P@V by 2; V proj and
the K proj of later pairs stream inside the first q-chunk's exp windows;
o_proj of qc-1 and Q-proj of qc+1 hide under the later exp windows.
"""
import os
import sys

sys.path.insert(0, "/opt/trn_rl_repo")

import numpy as np
import ml_dtypes

import concourse.bass as bass
import concourse.mybir as mybir
import concourse.tile as tile
from concourse import library_config
from concourse.bass_utils import run_bass_kernel_spmd
from concourse.vector_clock import ScopedClock, VectorClock

# ---------------------------------------------------------------------------
# Patch TileContext._drain_and_barrier: the walrus build in this container
# allows only ONE sync-wait per instruction; Tile's tail drain carries one
# wait per active proc.  Split them into single-wait NOPs on SP.
N_PROCS = 27


def _patched_drain_and_barrier(self, tick_clock, wait_clock):
    nc = self.nc
    gc = tick_clock.global_clock
    for p in range(N_PROCS):
        t = gc[p]
        if t > 0:
            nop = nc.sync.nop(nofuse=True)
            vc = VectorClock([t if q == p else 0 for q in range(N_PROCS)])
            wait_clock.add_sem_waits(nop.ins, ScopedClock({None: vc}))
    nc.sync.drain()
    nc.all_engine_barrier()
    assert self.sems is not None
    popped = nc._tile_sem_poison_stack.pop()
    assert popped is self._sem_poison
    nc.clear_and_free_semaphores(list(self.sems.allocated().values()))
    nc.all_engine_barrier()


tile.TileContext._drain_and_barrier = _patched_drain_and_barrier


def _split_excess_waits(nc):
    """walrus in this container accepts 1 sync-wait per instruction (2 on
    EventSemaphore).  Move excess waits onto EventSemaphore instructions
    inserted just before, on the same engine."""
    for f in nc.m.functions:
        for bb in f.blocks:
            new_insts = []
            changed = False
            for ins in bb.instructions:
                si = ins.sync_info
                waits = list(si.on_wait) if si is not None else []
                cap = 2 if isinstance(ins, mybir.InstEventSemaphore) else 1
                if len(waits) > cap:
                    changed = True
                    excess = waits[: len(waits) - cap]
                    for i in range(0, len(excess), 2):
                        ev = mybir.InstEventSemaphore(
                            name=f"I-{nc.next_id()}",
                            engine=ins.engine,
                            ins=[],
                            outs=[],
                            sync_info=mybir.SyncInfo(
                                on_wait=excess[i : i + 2], on_update=[]
                            ),
                        )
                        nc.register_instruction(ev)
                        new_insts.append(ev)
                    si.on_wait = waits[len(waits) - cap :]
                new_insts.append(ins)
            if changed:
                bb.instructions[:] = new_insts
# ---------------------------------------------------------------------------

B, S, H, NH, HD = 4, 2048, 1024, 16, 64
HPC = NH // 2          # heads per core
PAIRS = HPC // 2       # head pairs per core
HT = H // 128          # hidden-dim tiles
QKF = 2 * HPC * HD     # q+k features per core (1024)
VF = HPC * HD          # v features per core (512)
SC = 512               # seq chunk (psum bank)
NSC = S // SC
KT = S // 128          # kpos tiles
QT = SC // 128         # q sub-tiles per chunk
BF = mybir.dt.float16  # fp16: 10-bit mantissa, same PE/DVE speed as bf16
F32 = mybir.dt.float32
EXP_SCALE = 1.0 / float(np.sqrt(HD))

# ---------------------------------------------------------------------------
# Stream order is (pair, qc, kt): each pair runs its four q-chunk windows
# back to back, so the K/V/Q projections for pair p+1 spread across pair p's
# four exp windows instead of all crowding into the first q-chunk.
# Hook schedule per window (pair, qc): kt -> list of work items; kt 12-15 are
# kept hook-free so the DVE normalize and the scores leads at each window
# boundary are never queued behind hook work.
#   ("vo", st): V slice for this pair (own), two iterations ahead of use
#   ("vp", st): V slice prefetch for pair+1
#   ("k", c) / ("qn",) / ("qs", c): K chunk / Q c0 for pair+1, own Q chunk c
#   ("o", m): o_proj m-tile of chunk qc-1 (pair 3 windows only)
_HOOKS = {}
for _qc in range(4):
    for _kt in range(16):
        _HOOKS[(_qc, _kt)] = []
for _kt in range(12):  # own V slices, st 2..13 then 14/15 doubled at 10/11
    _HOOKS[(0, _kt)].append(("vo", _kt + 2))
_HOOKS[(0, 10)].append(("vo", 14))
_HOOKS[(0, 11)].append(("vo", 15))
for _i, _st in enumerate(range(16)):  # prefetch pair+1 V across qc1-3
    _qc = 1 + _i // 6
    _HOOKS[(_qc, (_i % 6) * 2)].append(("vp", _st))
_HOOKS[(1, 3)].append(("k", 0))
_HOOKS[(1, 9)].append(("k", 1))
_HOOKS[(2, 3)].append(("k", 2))
_HOOKS[(2, 9)].append(("k", 3))
_HOOKS[(3, 3)].append(("qn",))
for _qc in range(3):  # own q chunk qc+1
    _HOOKS[(_qc, 5)].append(("qs", _qc + 1))
# o_proj slots in pair-3 windows: first at kt3 so the previous chunk's
# pair-3 XBAR transpose (~2.9us) has landed; none at kt 13-15.
_OPROJ_KTS = (3, 4, 6, 7, 9, 10, 11, 12)
# ---------------------------------------------------------------------------

_CACHED_NC = None


def _build_nc():
    nc = bass.Bass()
    hT = nc.declare_dram_parameter("hT", [128, HT, S], BF, isOutput=False)
    # wqk feature order (host-packed): [k_p0, q_p0, k_p1, q_p1, ...] so the
    # first 256 columns are everything pair 0 needs to start.
    wqk = nc.declare_dram_parameter("wqk", [128, HT, QKF], BF, isOutput=False)
    wv = nc.declare_dram_parameter("wv", [128, HT, VF], BF, isOutput=False)
    wo = nc.declare_dram_parameter("wo", [128, VF // 128, H], BF, isOutput=False)
    cos_t = nc.declare_dram_parameter("cos_t", [128, S], BF, isOutput=False)
    sin_t = nc.declare_dram_parameter("sin_t", [128, S], BF, isOutput=False)
    outT = nc.declare_dram_parameter("outT", [H, S], F32, isOutput=True)

    Exp = mybir.ActivationFunctionType.Exp

    with tile.TileContext(nc) as tc:
        with tc.tile_pool(name="singles", bufs=1) as singles:
            hT_sb = singles.tile([128, HT, S], BF)
            wqk_sb = singles.tile([128, HT, QKF], BF)
            wv_sb = singles.tile([128, HT, VF], BF)
            wo_sb = singles.tile([128, VF // 128, H], BF)
            cos_sb = singles.tile([128, S], BF)
            sin_sb = singles.tile([128, S], BF)
            q_rope = singles.tile([128, PAIRS, S], BF)
            k_rope = singles.tile([128, PAIRS, S], BF)
            vext = singles.tile([128, KT, HPC * 65], BF)

            # DMA priority order: pair-0 weights first, then hidden, rope
            # tables, V weights, remaining QKV weights, o_proj weights.
            for k in range(HT):
                nc.sync.dma_start(out=wqk_sb[:, k, 0:256], in_=wqk[:, k, 0:256])
            for k in range(HT):
                nc.sync.dma_start(out=hT_sb[:, k, :], in_=hT[:, k, :])
            nc.sync.dma_start(out=cos_sb[:], in_=cos_t[:])
            nc.sync.dma_start(out=sin_sb[:], in_=sin_t[:])
            for k in range(HT):
                nc.sync.dma_start(out=wv_sb[:, k, :], in_=wv[:, k, :])
            for k in range(HT):
                nc.sync.dma_start(out=wqk_sb[:, k, 256:QKF], in_=wqk[:, k, 256:QKF])
            nc.sync.dma_start(out=wo_sb[:], in_=wo[:])
            nc.gpsimd.memset(vext[:], 1.0)

            # ---- pools (PSUM: sps 4 + pvps 2 + projps 1 + opps 1 = 8) ----
            with (
                tc.tile_pool(name="sps", bufs=2, space="PSUM") as sps,
                tc.tile_pool(name="pvps", bufs=1, space="PSUM") as pvps,
                tc.tile_pool(name="projps", bufs=1, space="PSUM") as projps,
                tc.tile_pool(name="opps", bufs=1, space="PSUM") as opps,
                tc.tile_pool(name="raws", bufs=3) as raws,
                tc.tile_pool(name="ropet", bufs=3) as ropet,
                tc.tile_pool(name="eps", bufs=6) as eps,
                tc.tile_pool(name="recs", bufs=4) as recs,
                tc.tile_pool(name="attns", bufs=4) as attns,
                tc.tile_pool(name="attnTs", bufs=4) as attnTs,
                tc.tile_pool(name="obs", bufs=4) as obs,
            ):

                def alloc_ps(pool):
                    """[128, SC] f32 psum AP from pool, one tag per pool so
                    every pool stays single-slot (1 bank; sps slots 2 banks)."""
                    if pool is sps:
                        s2t = sps.tile([128, 2, SC], F32, tag="s2", name="s2t")
                        return s2t[:, 0, :]
                    if pool is projps:
                        return projps.tile([128, SC], F32, tag="pj", name="pj")[:]
                    return opps.tile([128, SC], F32, tag="op", name="op")[:]

                def proj_chunk(m, c, psum_pool=None, copy_eng="vector"):
                    """Project q/k feature tile m (pair m//2, k if m even else
                    q) for seq chunk c, apply RoPE."""
                    pair = m // 2
                    dst_t = k_rope if m % 2 == 0 else q_rope
                    ps = alloc_ps(sps if psum_pool is None else psum_pool)
                    for k in range(HT):
                        nc.tensor.matmul(
                            ps,
                            wqk_sb[:, k, m * 128 : (m + 1) * 128],
                            hT_sb[:, k, c * SC : (c + 1) * SC],
                            start=(k == 0),
                            stop=(k == HT - 1),
                        )
                    raw = raws.tile([128, SC], BF)
                    if copy_eng == "vector":
                        nc.vector.tensor_copy(raw[:], ps)
                    else:
                        nc.scalar.copy(raw[:], ps)
                    cs = cos_sb[:, c * SC : (c + 1) * SC]
                    sn = sin_sb[:, c * SC : (c + 1) * SC]
                    dst = dst_t[:, pair, c * SC : (c + 1) * SC]
                    t1 = ropet.tile([128, SC], BF, tag="t1")
                    t2 = ropet.tile([128, SC], BF, tag="t2")
                    nc.vector.tensor_mul(t1[:], raw[:], cs)
                    nc.vector.tensor_mul(t2[0:32], raw[32:64], sn[32:64])
                    nc.vector.tensor_mul(t2[32:64], raw[0:32], sn[0:32])
                    nc.vector.tensor_mul(t2[64:96], raw[96:128], sn[96:128])
                    nc.vector.tensor_mul(t2[96:128], raw[64:96], sn[64:96])
                    nc.vector.tensor_add(dst, t1[:], t2[:])

                def v_proj(st, pair, psum_pool, copy_eng="vector"):
                    """V slice for one head pair (128 features) of kpos tile
                    st; cheap (128 moving cols) so it never bursts the PE."""
                    ps = alloc_ps(psum_pool)
                    fsl = slice(pair * 128, (pair + 1) * 128)
                    for k in range(HT):
                        nc.tensor.matmul(
                            ps[:, 0:128],
                            hT_sb[:, k, st * 128 : (st + 1) * 128],
                            wv_sb[:, k, fsl],
                            start=(k == 0),
                            stop=(k == HT - 1),
                        )
                    vdst = vext[:, st, :].rearrange("p (h x) -> p h x", x=65)[
                        :, 2 * pair : 2 * pair + 2, 0:64
                    ]
                    vsrc = ps[:, 0:128].rearrange("p (h x) -> p h x", x=64)
                    if copy_eng == "vector":
                        nc.vector.tensor_copy(vdst, vsrc)
                    else:
                        nc.scalar.copy(vdst, vsrc)

                def emit_scores(pair, kt, qsl):
                    ksl = slice(kt * 128, (kt + 1) * 128)
                    s2 = sps.tile([128, 2, SC], F32, tag="s2", name="s2")
                    nc.tensor.matmul(
                        s2[:, 0, :],
                        k_rope[0:64, pair, ksl],
                        q_rope[0:64, pair, qsl],
                        start=True,
                        stop=True,
                        tile_position=(0, 0),
                    )
                    nc.tensor.matmul(
                        s2[:, 1, :],
                        k_rope[64:128, pair, ksl],
                        q_rope[64:128, pair, qsl],
                        start=True,
                        stop=True,
                        tile_position=(64, 0),
                    )
                    return s2

                def o_proj_m(qc, m, attnT_q, pool):
                    qsl = slice(qc * SC, (qc + 1) * SC)
                    op = alloc_ps(pool)
                    for ot in range(VF // 128):
                        nc.tensor.matmul(
                            op,
                            wo_sb[:, ot, m * 128 : (m + 1) * 128],
                            attnT_q[:, ot, :],
                            start=(ot == 0),
                            stop=(ot == VF // 128 - 1),
                        )
                    ob = obs.tile([128, SC], F32, tag="ob", name="ob")
                    nc.vector.tensor_copy(ob[:], op)
                    nc.sync.dma_start(out=outT[m * 128 : (m + 1) * 128, qsl], in_=ob[:])

                # ---- head phase: pair-0 K c0 and Q c0 first (they gate the
                # first exp), then the rest of pair-0 K and two V slices.
                # ACT handles the psum->sbuf copies here (it is idle until the
                # first exp); DVE handles them once the exp stream runs.
                proj_chunk(0, 0, copy_eng="scalar")          # k pair0 c0
                proj_chunk(1, 0, copy_eng="scalar")          # q pair0 c0
                for c in range(1, NSC):
                    proj_chunk(0, c, copy_eng="scalar")      # k pair0 c1-3
                v_proj(0, 0, sps, copy_eng="scalar")
                v_proj(1, 0, sps, copy_eng="scalar")

                # ---- globally software-pipelined attention stream ----
                flat = [
                    (p, qc, k)
                    for p in range(PAIRS)
                    for qc in range(NSC)
                    for k in range(KT)
                ]

                def qsl_of(qc):
                    return slice(qc * SC, (qc + 1) * SC)

                hookn = [0]

                def hook(pair, qc, kt, attnT_tiles):
                    """PE/DMA producer work interleaved into iteration
                    (pair, qc, kt), emitted before the scores lead.  Pool
                    choice alternates globally so consecutive hook items
                    never reuse the same psum bank back to back."""
                    def next_pool():
                        pool = opps if hookn[0] % 2 == 0 else projps
                        hookn[0] += 1
                        return pool

                    for item in _HOOKS[(qc, kt)]:
                        kind = item[0]
                        if kind == "vo":
                            if pair == 0:
                                v_proj(item[1], 0, next_pool())
                        elif kind == "vp":
                            if pair < PAIRS - 1:
                                v_proj(item[1], pair + 1, next_pool())
                        elif kind == "k":
                            if pair < PAIRS - 1:
                                proj_chunk(2 * (pair + 1), item[1], next_pool())
                        elif kind == "qn":
                            if pair < PAIRS - 1:
                                proj_chunk(2 * (pair + 1) + 1, 0, next_pool())
                        elif kind == "qs":
                            # pair 3's own-q rope would queue on DVE mid
                            # o_proj train; it runs at kt0 instead (below)
                            if pair < PAIRS - 1:
                                proj_chunk(2 * pair + 1, item[1], next_pool())
                    if pair == PAIRS - 1 and kt == 0 and qc < NSC - 1:
                        proj_chunk(2 * pair + 1, qc + 1, next_pool())
                    if pair == PAIRS - 1 and qc >= 1 and kt in _OPROJ_KTS:
                        m = _OPROJ_KTS.index(kt)
                        o_proj_m(qc - 1, m, attnT_tiles[qc - 1], next_pool())

                def pv_mms(pair, qc, kt, e, heads):
                    # acc banks hold four 65-col accumulation groups each; a
                    # start=True would zero the whole bank and wipe sibling
                    # groups, so the accs are memset-zeroed at kt0 and every
                    # matmul is a pure accumulate.
                    accA, accB = accs[(pair, qc)]
                    vx = vext[:, kt, :].rearrange("p (h x) -> p h x", x=65)
                    for h in heads:
                        acc = accA if h == 0 else accB
                        for qs in range(QT):
                            nc.tensor.matmul(
                                acc[:, qs, :],
                                e[:, h, qs * 128 : (qs + 1) * 128],
                                vx[:, 2 * pair + h, :],
                                start=False,
                                stop=(kt == KT - 1),
                                skip_group_check=True,
                            )

                s2q = {
                    0: emit_scores(flat[0][0], flat[0][2], qsl_of(flat[0][1])),
                    1: emit_scores(flat[1][0], flat[1][2], qsl_of(flat[1][1])),
                }
                accs = {}
                attn_tiles = {}
                attnT_tiles = {}
                ekt = {}
                for i, (pair, qc, kt) in enumerate(flat):
                    if pair == 0 and kt == 0:
                        attn_tiles[qc] = attns.tile(
                            [128, PAIRS, QT, 128], BF, tag="attn", name="attn"
                        )
                    if kt == 0:
                        accA = pvps.tile([128, QT, 65], F32, tag="acA", name="acA")
                        accB = pvps.tile([128, QT, 65], F32, tag="acB", name="acB")
                        accs[(pair, qc)] = (accA, accB)
                        nc.vector.memset(accA[:], 0.0)
                        nc.vector.memset(accB[:], 0.0)
                    hook(pair, qc, kt, attnT_tiles)
                    s2 = s2q.pop(i)
                    e = eps.tile([128, 2, SC], BF)
                    nc.scalar.activation(out=e[:], in_=s2[:], func=Exp, scale=EXP_SCALE)
                    if i + 2 < len(flat):
                        npair, nqc, nkt = flat[i + 2]
                        s2q[i + 2] = emit_scores(npair, nkt, qsl_of(nqc))
                    # P@V lags exp: head A by one iteration, head B by two.
                    # kt0's P@V waits on the previous window's normalize (psum
                    # WAR); the lag absorbs that latency and keeps any parked
                    # group within the 4-deep PE wait queue.
                    ekt[kt] = e
                    if kt >= 1:
                        pv_mms(pair, qc, kt - 1, ekt[kt - 1], (0,))
                    if kt >= 2:
                        pv_mms(pair, qc, kt - 2, ekt[kt - 2], (1,))
                    if kt == KT - 1:
                        pv_mms(pair, qc, KT - 1, ekt[KT - 1], (0,))
                        pv_mms(pair, qc, KT - 2, ekt[KT - 2], (1,))
                        pv_mms(pair, qc, KT - 1, ekt[KT - 1], (1,))
                        rec = recs.tile([128, 2, QT], F32)
                        accA, accB = accs.pop((pair, qc))
                        nc.vector.reciprocal(rec[:, 0, :], accA[:, :, 64])
                        nc.vector.reciprocal(rec[:, 1, :], accB[:, :, 64])
                        at = attn_tiles[qc]
                        for h, acc in ((0, accA), (1, accB)):
                            for qs in range(QT):
                                nc.vector.tensor_scalar_mul(
                                    at[:, pair, qs, h * 64 : (h + 1) * 64],
                                    acc[:, qs, 0:64],
                                    rec[:, h, qs : qs + 1],
                                )
                        # transpose this pair's attn [q, d] -> [d, q] in ONE
                        # XBAR DMA (no PE/PSUM); attnT(qc) complete once the
                        # pair-3 transpose lands
                        if pair == 0:
                            attnT_tiles[qc] = attnTs.tile(
                                [128, PAIRS, SC], BF, tag="aT", name="aT"
                            )
                        nc.sync.dma_start_transpose(
                            out=attnT_tiles[qc][:, pair, :].rearrange(
                                "p (di m) -> p di m", m=128
                            ),
                            in_=at[:, pair, :, :],
                        )

                # ---- tail: last chunk's o_proj.  Six psum slots (opps,
                # projps, and the now-idle scores banks) let the ot 0-2
                # matmuls prefill while the pair-3 transpose is in flight;
                # the ot-3 matmuls and copies then stream without bank waits.
                attnT_q = attnT_tiles[NSC - 1]
                t_s2a = sps.tile([128, 2, SC], F32, tag="s2", name="t_s2a")
                t_s2b = sps.tile([128, 2, SC], F32, tag="s2", name="t_s2b")
                t_slots = [
                    alloc_ps(opps),
                    alloc_ps(projps),
                    t_s2a[:, 0, :],
                    t_s2a[:, 1, :],
                    t_s2b[:, 0, :],
                    t_s2b[:, 1, :],
                ]
                qsl = qsl_of(NSC - 1)

                def t_oproj_mms(m, ots):
                    op = t_slots[m % 6]
                    for ot in ots:
                        nc.tensor.matmul(
                            op,
                            wo_sb[:, ot, m * 128 : (m + 1) * 128],
                            attnT_q[:, ot, :],
                            start=(ot == 0),
                            stop=(ot == VF // 128 - 1),
                        )

                def t_oproj_fin(m):
                    t_oproj_mms(m, (3,))
                    ob = obs.tile([128, SC], F32, tag="ob", name="ob")
                    # ACT is idle after the last exp: share the tail copies
                    if m % 2 == 0:
                        nc.vector.tensor_copy(ob[:], t_slots[m % 6])
                    else:
                        nc.scalar.copy(ob[:], t_slots[m % 6])
                    nc.sync.dma_start(
                        out=outT[m * 128 : (m + 1) * 128, qsl], in_=ob[:]
                    )

                for m in range(6):
                    t_oproj_mms(m, (0, 1, 2))
                for m in range(6):
                    t_oproj_fin(m)
                for m in (6, 7):
                    t_oproj_mms(m, (0, 1, 2))
                    t_oproj_fin(m)
    _split_excess_waits(nc)
    return nc


def _prep_inputs(cos, sin, hidden_states, w_qkv, w_o):
    """Per-core host-side sharding/transpose/cast. Returns list of in_maps."""
    bf = np.float16
    cos = np.asarray(cos, np.float32)
    sin = np.asarray(sin, np.float32)
    hidden_states = np.asarray(hidden_states, np.float32)
    w_qkv = np.asarray(w_qkv, np.float32)
    w_o = np.asarray(w_o, np.float32)

    cosT = cos.T  # [64, S]
    cos_t = np.ascontiguousarray(np.tile(cosT, (2, 1))).astype(bf)
    # sin multiplier aligned to the *source* partitions of the rot ops:
    # rows [0:32] = +sin[32:64] (multiplies src q[0:32] -> dest [32:64]),
    # rows [32:64] = -sin[0:32] (multiplies src q[32:64] -> dest [0:32]).
    sinT = sin.T
    sin_t = np.ascontiguousarray(
        np.tile(np.concatenate([sinT[32:], -sinT[:32]], 0), (2, 1))
    ).astype(bf)

    in_maps = []
    for core in range(8):
        b, g = core // 2, core % 2
        hT = hidden_states[b].T  # [H, S]
        hT_t = np.ascontiguousarray(
            hT.reshape(HT, 128, S).transpose(1, 0, 2)
        ).astype(bf)
        qs, ks, vs = g * VF, NH * HD + g * VF, 2 * NH * HD + g * VF
        # interleave per pair: [k_p0, q_p0, k_p1, q_p1, ...]
        blocks = []
        for p in range(PAIRS):
            blocks.append(w_qkv[ks + p * 128 : ks + (p + 1) * 128])
            blocks.append(w_qkv[qs + p * 128 : qs + (p + 1) * 128])
        wqk_rows = np.concatenate(blocks, 0)  # [QKF, H]
        wqk_t = np.ascontiguousarray(
            wqk_rows.T.reshape(HT, 128, QKF).transpose(1, 0, 2)
        ).astype(bf)
        wv_t = np.ascontiguousarray(
            w_qkv[vs : vs + VF].T.reshape(HT, 128, VF).transpose(1, 0, 2)
        ).astype(bf)
        woT = w_o[:, g * VF : (g + 1) * VF].T  # [VF, H]
        wo_t = np.ascontiguousarray(
            woT.reshape(VF // 128, 128, H).transpose(1, 0, 2)
        ).astype(bf)
        in_maps.append(
            {
                "hT": hT_t,
                "wqk": wqk_t,
                "wv": wv_t,
                "wo": wo_t,
                "cos_t": cos_t,
                "sin_t": sin_t,
            }
        )
    return in_maps


def kernel(cos, sin, hidden_states, w_qkv, w_o, _trace=False):
    global _CACHED_NC
    if _CACHED_NC is None:
        _CACHED_NC = _build_nc()
    nc = _CACHED_NC
    in_maps = _prep_inputs(cos, sin, hidden_states, w_qkv, w_o)
    res = run_bass_kernel_spmd(nc, in_maps, core_ids=list(range(8)), trace=_trace)
    outs = [r["outT"] for r in res.results]
    out = np.empty((B, S, H), np.float32)
    for b in range(B):
        out[b] = (outs[2 * b] + outs[2 * b + 1]).T
    if _trace:
        return out, res
    return out


# revision 40
# speedup vs baseline: 1.2523x; 1.0178x over previous
"""Trainium2 Bass kernel for nn_Attention: fused QKV + RoPE + softmax attention + o_proj.

Sharding (8 cores): core c -> (batch b = c//2, head-half g = c%2).
Each core computes 8 of 16 heads for one batch:
  - QKV projection (fp16 matmuls, weights pre-transposed/tiled on host)
  - RoPE on DVE (d-on-partition layout, rotation = partition-block swap)
  - scores^T [kpos, q] per head pair, row-packed via tile_position
  - exp on ACT (1/sqrt(d) folded into the activation scale), fp16 out
  - P@V transposed: probs chunk is the *stationary* operand, V (64 dims +
    ones column = 65) is the moving operand -> out [128 q, 65] accumulated
    over kpos; column 64 is the softmax denominator
  - normalize on DVE with a per-partition reciprocal scalar (denominator
    now lives on the q partition), fp16 attn [q, d]
  - attn [q, d] -> [d, q] via DMA XBAR transpose (SBUF->SBUF, no PE/PSUM)
  - o_proj with w_o columns sharded by head; partial out^T [H, S] fp32;
    host sums the two head-half partials per batch
Emission is globally software-pipelined: scores lead exp/P@V by 2; V proj and
the K proj of later pairs stream inside the first q-chunk's exp windows;
o_proj of qc-1 and Q-proj of qc+1 hide under the later exp windows.
"""
import os
import sys

sys.path.insert(0, "/opt/trn_rl_repo")

import numpy as np
import ml_dtypes

import concourse.bass as bass
import concourse.mybir as mybir
import concourse.tile as tile
from concourse import library_config
from concourse.bass_utils import run_bass_kernel_spmd
from concourse.vector_clock import ScopedClock, VectorClock

# ---------------------------------------------------------------------------
# Patch TileContext._drain_and_barrier: the walrus build in this container
# allows only ONE sync-wait per instruction; Tile's tail drain carries one
# wait per active proc.  Split them into single-wait NOPs on SP.
N_PROCS = 27


def _patched_drain_and_barrier(self, tick_clock, wait_clock):
    nc = self.nc
    gc = tick_clock.global_clock
    for p in range(N_PROCS):
        t = gc[p]
        if t > 0:
            nop = nc.sync.nop(nofuse=True)
            vc = VectorClock([t if q == p else 0 for q in range(N_PROCS)])
            wait_clock.add_sem_waits(nop.ins, ScopedClock({None: vc}))
    nc.sync.drain()
    nc.all_engine_barrier()
    assert self.sems is not None
    popped = nc._tile_sem_poison_stack.pop()
    assert popped is self._sem_poison
    nc.clear_and_free_semaphores(list(self.sems.allocated().values()))
    nc.all_engine_barrier()


tile.TileContext._drain_and_barrier = _patched_drain_and_barrier


def _split_excess_waits(nc):
    """walrus in this container accepts 1 sync-wait per instruction (2 on
    EventSemaphore).  Move excess waits onto EventSemaphore instructions
    inserted just before, on the same engine."""
    for f in nc.m.functions:
        for bb in f.blocks:
            new_insts = []
            changed = False
            for ins in bb.instructions:
                si = ins.sync_info
                waits = list(si.on_wait) if si is not None else []
                cap = 2 if isinstance(ins, mybir.InstEventSemaphore) else 1
                if len(waits) > cap:
                    changed = True
                    excess = waits[: len(waits) - cap]
                    for i in range(0, len(excess), 2):
                        ev = mybir.InstEventSemaphore(
                            name=f"I-{nc.next_id()}",
                            engine=ins.engine,
                            ins=[],
                            outs=[],
                            sync_info=mybir.SyncInfo(
                                on_wait=excess[i : i + 2], on_update=[]
                            ),
                        )
                        nc.register_instruction(ev)
                        new_insts.append(ev)
                    si.on_wait = waits[len(waits) - cap :]
                new_insts.append(ins)
            if changed:
                bb.instructions[:] = new_insts
# ---------------------------------------------------------------------------

B, S, H, NH, HD = 4, 2048, 1024, 16, 64
HPC = NH // 2          # heads per core
PAIRS = HPC // 2       # head pairs per core
HT = H // 128          # hidden-dim tiles
QKF = 2 * HPC * HD     # q+k features per core (1024)
VF = HPC * HD          # v features per core (512)
SC = 512               # seq chunk (psum bank)
NSC = S // SC
KT = S // 128          # kpos tiles
QT = SC // 128         # q sub-tiles per chunk
BF = mybir.dt.float16  # fp16: 10-bit mantissa, same PE/DVE speed as bf16
F32 = mybir.dt.float32
EXP_SCALE = 1.0 / float(np.sqrt(HD))

# ---------------------------------------------------------------------------
# Stream order is (pair, qc, kt): each pair runs its four q-chunk windows
# back to back, so the K/V/Q projections for pair p+1 spread across pair p's
# four exp windows instead of all crowding into the first q-chunk.
# Hook schedule per window (pair, qc): kt -> list of work items; kt 12-15 are
# kept hook-free so the DVE normalize and the scores leads at each window
# boundary are never queued behind hook work.
#   ("vo", st): V slice for this pair (own), two iterations ahead of use
#   ("vp", st): V slice prefetch for pair+1
#   ("k", c) / ("qn",) / ("qs", c): K chunk / Q c0 for pair+1, own Q chunk c
#   ("o", m): o_proj m-tile of chunk qc-1 (pair 3 windows only)
_HOOKS = {}
for _qc in range(4):
    for _kt in range(16):
        _HOOKS[(_qc, _kt)] = []
for _kt in range(12):  # own V slices, st 2..13 then 14/15 doubled at 10/11
    _HOOKS[(0, _kt)].append(("vo", _kt + 2))
_HOOKS[(0, 10)].append(("vo", 14))
_HOOKS[(0, 11)].append(("vo", 15))
for _i, _st in enumerate(range(16)):  # prefetch pair+1 V across qc1-3
    _qc = 1 + _i // 6
    _HOOKS[(_qc, (_i % 6) * 2)].append(("vp", _st))
_HOOKS[(1, 3)].append(("k", 0))
_HOOKS[(1, 9)].append(("k", 1))
_HOOKS[(2, 3)].append(("k", 2))
_HOOKS[(2, 9)].append(("k", 3))
_HOOKS[(3, 3)].append(("qn",))
for _qc in range(3):  # own q chunk qc+1
    _HOOKS[(_qc, 5)].append(("qs", _qc + 1))
# o_proj slots in pair-3 windows: first at kt3 so the previous chunk's
# pair-3 XBAR transpose (~2.9us) has landed; none at kt 13-15.
_OPROJ_KTS = (3, 4, 6, 7, 9, 10, 11, 12)
# ---------------------------------------------------------------------------

_CACHED_NC = None


def _build_nc():
    nc = bass.Bass()
    hT = nc.declare_dram_parameter("hT", [128, HT, S], BF, isOutput=False)
    # wqk feature order (host-packed): [k_p0, q_p0, k_p1, q_p1, ...] so the
    # first 256 columns are everything pair 0 needs to start.
    wqk = nc.declare_dram_parameter("wqk", [128, HT, QKF], BF, isOutput=False)
    wv = nc.declare_dram_parameter("wv", [128, HT, VF], BF, isOutput=False)
    wo = nc.declare_dram_parameter("wo", [128, VF // 128, H], BF, isOutput=False)
    cos_t = nc.declare_dram_parameter("cos_t", [128, S], BF, isOutput=False)
    sin_t = nc.declare_dram_parameter("sin_t", [128, S], BF, isOutput=False)
    outT = nc.declare_dram_parameter("outT", [H, S], F32, isOutput=True)

    Exp = mybir.ActivationFunctionType.Exp

    with tile.TileContext(nc) as tc:
        with tc.tile_pool(name="singles", bufs=1) as singles:
            hT_sb = singles.tile([128, HT, S], BF)
            wqk_sb = singles.tile([128, HT, QKF], BF)
            wv_sb = singles.tile([128, HT, VF], BF)
            wo_sb = singles.tile([128, VF // 128, H], BF)
            cos_sb = singles.tile([128, S], BF)
            sin_sb = singles.tile([128, S], BF)
            q_rope = singles.tile([128, PAIRS, S], BF)
            k_rope = singles.tile([128, PAIRS, S], BF)
            vext = singles.tile([128, KT, HPC * 65], BF)
            zeros_sb = singles.tile([128, QT * 65], BF)
            nc.vector.memset(zeros_sb[:], 0.0)

            # DMA priority order: pair-0 weights first, then hidden, rope
            # tables, V weights, remaining QKV weights, o_proj weights.
            # DMA priority: pair-0 q/k weights (one fused transfer), hidden
            # in 4 fragments (the first projection chains track arrival),
            # chunk-0 rope tables, V weights, remaining rope chunks, the
            # rest of the QKV weights, and o_proj weights.  Transfers are
            # fused where the consumer granularity allows: the HWDGE setup
            # (~630ns each) otherwise delays the critical hT stream.
            nc.sync.dma_start(out=wqk_sb[:, :, 0:256], in_=wqk[:, :, 0:256])
            for g in range(4):
                ksl = slice(2 * g, 2 * g + 2)
                nc.sync.dma_start(out=hT_sb[:, ksl, :], in_=hT[:, ksl, :])
            nc.sync.dma_start(out=cos_sb[:, 0:SC], in_=cos_t[:, 0:SC])
            nc.sync.dma_start(out=sin_sb[:, 0:SC], in_=sin_t[:, 0:SC])
            nc.sync.dma_start(out=wv_sb[:], in_=wv[:])
            for c in range(1, NSC):
                csl = slice(c * SC, (c + 1) * SC)
                nc.sync.dma_start(out=cos_sb[:, csl], in_=cos_t[:, csl])
                nc.sync.dma_start(out=sin_sb[:, csl], in_=sin_t[:, csl])
            nc.sync.dma_start(out=wqk_sb[:, :, 256:QKF], in_=wqk[:, :, 256:QKF])
            nc.sync.dma_start(out=wo_sb[:], in_=wo[:])
            nc.gpsimd.memset(vext[:], 1.0)

            # ---- pools (PSUM: sps 4 + pvps 2 + projps 1 + opps 1 = 8) ----
            with (
                tc.tile_pool(name="sps", bufs=2, space="PSUM") as sps,
                tc.tile_pool(name="pvps", bufs=1, space="PSUM") as pvps,
                tc.tile_pool(name="projps", bufs=1, space="PSUM") as projps,
                tc.tile_pool(name="opps", bufs=1, space="PSUM") as opps,
                tc.tile_pool(name="raws", bufs=3) as raws,
                tc.tile_pool(name="ropet", bufs=3) as ropet,
                tc.tile_pool(name="eps", bufs=6) as eps,
                tc.tile_pool(name="recs", bufs=4) as recs,
                tc.tile_pool(name="attns", bufs=4) as attns,
                tc.tile_pool(name="attnTs", bufs=4) as attnTs,
                tc.tile_pool(name="obs", bufs=4) as obs,
            ):

                def alloc_ps(pool):
                    """[128, SC] f32 psum AP from pool, one tag per pool so
                    every pool stays single-slot (1 bank; sps slots 2 banks)."""
                    if pool is sps:
                        s2t = sps.tile([128, 2, SC], F32, tag="s2", name="s2t")
                        return s2t[:, 0, :]
                    if pool is projps:
                        return projps.tile([128, SC], F32, tag="pj", name="pj")[:]
                    return opps.tile([128, SC], F32, tag="op", name="op")[:]

                def rope_apply(raw, m, c, off, ln):
                    """RoPE on columns [off, off+ln) of chunk c of feature
                    tile m, from the fp16 raw tile into q_rope/k_rope."""
                    pair = m // 2
                    dst_t = k_rope if m % 2 == 0 else q_rope
                    lo, hi = c * SC + off, c * SC + off + ln
                    cs = cos_sb[:, lo:hi]
                    sn = sin_sb[:, lo:hi]
                    dst = dst_t[:, pair, lo:hi]
                    r = raw[:, off : off + ln]
                    t1 = ropet.tile([128, SC], BF, tag="t1")
                    t2 = ropet.tile([128, SC], BF, tag="t2")
                    t1 = t1[:, 0:ln]
                    t2 = t2[:, 0:ln]
                    nc.vector.tensor_mul(t1, r, cs)
                    nc.vector.tensor_mul(t2[0:32], r[32:64], sn[32:64])
                    nc.vector.tensor_mul(t2[32:64], r[0:32], sn[0:32])
                    nc.vector.tensor_mul(t2[64:96], r[96:128], sn[96:128])
                    nc.vector.tensor_mul(t2[96:128], r[64:96], sn[64:96])
                    nc.vector.tensor_add(dst, t1, t2)

                def proj_chunk(m, c, psum_pool=None, copy_eng="vector"):
                    """Project q/k feature tile m (pair m//2, k if m even else
                    q) for seq chunk c, apply RoPE."""
                    ps = alloc_ps(sps if psum_pool is None else psum_pool)
                    for k in range(HT):
                        nc.tensor.matmul(
                            ps,
                            wqk_sb[:, k, m * 128 : (m + 1) * 128],
                            hT_sb[:, k, c * SC : (c + 1) * SC],
                            start=(k == 0),
                            stop=(k == HT - 1),
                        )
                    raw = raws.tile([128, SC], BF)
                    if copy_eng == "vector":
                        nc.vector.tensor_copy(raw[:], ps)
                    else:
                        nc.scalar.copy(raw[:], ps)
                    rope_apply(raw, m, c, 0, SC)

                def v_proj(st, pair, psum_pool, copy_eng="vector"):
                    """V slice for one head pair (128 features) of kpos tile
                    st; cheap (128 moving cols) so it never bursts the PE."""
                    ps = alloc_ps(psum_pool)
                    fsl = slice(pair * 128, (pair + 1) * 128)
                    for k in range(HT):
                        nc.tensor.matmul(
                            ps[:, 0:128],
                            hT_sb[:, k, st * 128 : (st + 1) * 128],
                            wv_sb[:, k, fsl],
                            start=(k == 0),
                            stop=(k == HT - 1),
                        )
                    vdst = vext[:, st, :].rearrange("p (h x) -> p h x", x=65)[
                        :, 2 * pair : 2 * pair + 2, 0:64
                    ]
                    vsrc = ps[:, 0:128].rearrange("p (h x) -> p h x", x=64)
                    if copy_eng == "vector":
                        nc.vector.tensor_copy(vdst, vsrc)
                    else:
                        nc.scalar.copy(vdst, vsrc)

                def emit_scores(pair, kt, qsl):
                    ksl = slice(kt * 128, (kt + 1) * 128)
                    s2 = sps.tile([128, 2, SC], F32, tag="s2", name="s2")
                    nc.tensor.matmul(
                        s2[:, 0, :],
                        k_rope[0:64, pair, ksl],
                        q_rope[0:64, pair, qsl],
                        start=True,
                        stop=True,
                        tile_position=(0, 0),
                    )
                    nc.tensor.matmul(
                        s2[:, 1, :],
                        k_rope[64:128, pair, ksl],
                        q_rope[64:128, pair, qsl],
                        start=True,
                        stop=True,
                        tile_position=(64, 0),
                    )
                    return s2

                def o_proj_m(qc, m, attnT_q, pool):
                    qsl = slice(qc * SC, (qc + 1) * SC)
                    op = alloc_ps(pool)
                    for ot in range(VF // 128):
                        nc.tensor.matmul(
                            op,
                            wo_sb[:, ot, m * 128 : (m + 1) * 128],
                            attnT_q[:, ot, :],
                            start=(ot == 0),
                            stop=(ot == VF // 128 - 1),
                        )
                    ob = obs.tile([128, SC], F32, tag="ob", name="ob")
                    nc.vector.tensor_copy(ob[:], op)
                    nc.sync.dma_start(out=outT[m * 128 : (m + 1) * 128, qsl], in_=ob[:])

                # ---- head phase.  The first exp is gated by k-c0/q-c0 of
                # pair 0: run both projection chains k-major (so they track
                # the hT fragment arrivals), copy on ACT (idle until the
                # first exp), rope q whole but k in 128-col slices so the
                # first scores only wait on the first slice.
                hg = sps.tile([128, 2, SC], F32, tag="s2", name="hg")
                for k in range(HT):
                    for j in range(2):
                        nc.tensor.matmul(
                            hg[:, j, :],
                            wqk_sb[:, k, j * 128 : (j + 1) * 128],
                            hT_sb[:, k, 0:SC],
                            start=(k == 0),
                            stop=(k == HT - 1),
                        )
                raw_k = raws.tile([128, SC], BF)
                raw_q = raws.tile([128, SC], BF)
                nc.scalar.copy(raw_q[:], hg[:, 1, :])
                nc.scalar.copy(raw_k[:], hg[:, 0, :])
                rope_apply(raw_q, 1, 0, 0, SC)               # q pair0 c0
                for sl in range(QT):
                    rope_apply(raw_k, 0, 0, sl * 128, 128)   # k pair0 c0
                for c in range(1, NSC):
                    proj_chunk(0, c, copy_eng="scalar")      # k pair0 c1-3
                v_proj(0, 0, sps, copy_eng="scalar")
                v_proj(1, 0, sps, copy_eng="scalar")

                # ---- globally software-pipelined attention stream ----
                flat = [
                    (p, qc, k)
                    for p in range(PAIRS)
                    for qc in range(NSC)
                    for k in range(KT)
                ]

                def qsl_of(qc):
                    return slice(qc * SC, (qc + 1) * SC)

                hookn = [0]

                def hook(pair, qc, kt, attnT_tiles):
                    """PE/DMA producer work interleaved into iteration
                    (pair, qc, kt), emitted before the scores lead.  Pool
                    choice alternates globally so consecutive hook items
                    never reuse the same psum bank back to back."""
                    def next_pool():
                        pool = opps if hookn[0] % 2 == 0 else projps
                        hookn[0] += 1
                        return pool

                    for item in _HOOKS[(qc, kt)]:
                        kind = item[0]
                        if kind == "vo":
                            if pair == 0:
                                v_proj(item[1], 0, next_pool())
                        elif kind == "vp":
                            if pair < PAIRS - 1:
                                v_proj(item[1], pair + 1, next_pool())
                        elif kind == "k":
                            if pair < PAIRS - 1:
                                proj_chunk(2 * (pair + 1), item[1], next_pool())
                        elif kind == "qn":
                            if pair < PAIRS - 1:
                                proj_chunk(2 * (pair + 1) + 1, 0, next_pool())
                        elif kind == "qs":
                            # pair 3's own q chunks are prefetched from the
                            # pair-1/2 groups so its o_proj windows stay light
                            if pair < PAIRS - 1:
                                proj_chunk(2 * pair + 1, item[1], next_pool())
                    if pair == 1 and kt == 1 and qc in (1, 2):
                        proj_chunk(7, qc, next_pool())       # q pair3 c1/c2
                    if pair == 2 and kt == 1 and qc == 1:
                        proj_chunk(7, 3, next_pool())        # q pair3 c3
                    if pair == PAIRS - 1 and qc >= 1 and kt in _OPROJ_KTS:
                        m = _OPROJ_KTS.index(kt)
                        o_proj_m(qc - 1, m, attnT_tiles[qc - 1], next_pool())

                def pv_mms(pair, qc, kt, e, heads):
                    # acc banks hold four 65-col accumulation groups each; a
                    # start=True would zero the whole bank and wipe sibling
                    # groups, so the accs are memset-zeroed at kt0 and every
                    # matmul is a pure accumulate.
                    accA, accB = accs[(pair, qc)]
                    vx = vext[:, kt, :].rearrange("p (h x) -> p h x", x=65)
                    for h in heads:
                        acc = accA if h == 0 else accB
                        for qs in range(QT):
                            nc.tensor.matmul(
                                acc[:, qs, :],
                                e[:, h, qs * 128 : (qs + 1) * 128],
                                vx[:, 2 * pair + h, :],
                                start=False,
                                stop=(kt == KT - 1),
                                skip_group_check=True,
                            )

                s2q = {
                    0: emit_scores(flat[0][0], flat[0][2], qsl_of(flat[0][1])),
                    1: emit_scores(flat[1][0], flat[1][2], qsl_of(flat[1][1])),
                }
                accs = {}
                attn_tiles = {}
                attnT_tiles = {}
                ekt = {}
                for i, (pair, qc, kt) in enumerate(flat):
                    if pair == 0 and kt == 0:
                        attn_tiles[qc] = attns.tile(
                            [128, PAIRS, QT, 128], BF, tag="attn", name="attn"
                        )
                    if kt == 0:
                        accA = pvps.tile([128, QT, 65], F32, tag="acA", name="acA")
                        accB = pvps.tile([128, QT, 65], F32, tag="acB", name="acB")
                        accs[(pair, qc)] = (accA, accB)
                        nc.vector.memset(accA[:], 0.0)
                        nc.vector.memset(accB[:], 0.0)
                    hook(pair, qc, kt, attnT_tiles)
                    s2 = s2q.pop(i)
                    e = eps.tile([128, 2, SC], BF)
                    nc.scalar.activation(out=e[:], in_=s2[:], func=Exp, scale=EXP_SCALE)
                    if i + 2 < len(flat):
                        npair, nqc, nkt = flat[i + 2]
                        s2q[i + 2] = emit_scores(npair, nkt, qsl_of(nqc))
                    # P@V lags exp: head A by one iteration, head B by two.
                    # kt0's P@V waits on the previous window's normalize (psum
                    # WAR); the lag absorbs that latency and keeps any parked
                    # group within the 4-deep PE wait queue.
                    ekt[kt] = e
                    if kt >= 2:
                        pv_mms(pair, qc, kt - 2, ekt[kt - 2], (0,))
                    if kt >= 3:
                        pv_mms(pair, qc, kt - 3, ekt[kt - 3], (1,))
                    if kt == KT - 1:
                        pv_mms(pair, qc, KT - 2, ekt[KT - 2], (0,))
                        pv_mms(pair, qc, KT - 1, ekt[KT - 1], (0,))
                        pv_mms(pair, qc, KT - 3, ekt[KT - 3], (1,))
                        pv_mms(pair, qc, KT - 2, ekt[KT - 2], (1,))
                        pv_mms(pair, qc, KT - 1, ekt[KT - 1], (1,))
                        rec = recs.tile([128, 2, QT], F32)
                        accA, accB = accs[(pair, qc)]
                        at = attn_tiles[qc]
                        nc.vector.reciprocal(rec[:, 0, :], accA[:, :, 64])
                        nc.vector.reciprocal(rec[:, 1, :], accB[:, :, 64])
                        for h, acc in ((0, accA), (1, accB)):
                            for qs in range(QT):
                                nc.vector.tensor_scalar_mul(
                                    at[:, pair, qs, h * 64 : (h + 1) * 64],
                                    acc[:, qs, 0:64],
                                    rec[:, h, qs : qs + 1],
                                )
                        # transpose this pair's attn [q, d] -> [d, q] in ONE
                        # XBAR DMA (no PE/PSUM); attnT(qc) complete once the
                        # pair-3 transpose lands
                        if pair == 0:
                            attnT_tiles[qc] = attnTs.tile(
                                [128, PAIRS, SC], BF, tag="aT", name="aT"
                            )
                        nc.sync.dma_start_transpose(
                            out=attnT_tiles[qc][:, pair, :].rearrange(
                                "p (di m) -> p di m", m=128
                            ),
                            in_=at[:, pair, :, :],
                        )

                # ---- tail: last chunk's o_proj.  Six psum slots (opps,
                # projps, and the now-idle scores banks) let the ot 0-2
                # matmuls prefill while the pair-3 transpose is in flight;
                # the ot-3 matmuls and copies then stream without bank waits.
                attnT_q = attnT_tiles[NSC - 1]
                t_s2a = sps.tile([128, 2, SC], F32, tag="s2", name="t_s2a")
                t_s2b = sps.tile([128, 2, SC], F32, tag="s2", name="t_s2b")
                t_slots = [
                    alloc_ps(opps),
                    alloc_ps(projps),
                    t_s2a[:, 0, :],
                    t_s2a[:, 1, :],
                    t_s2b[:, 0, :],
                    t_s2b[:, 1, :],
                ]
                qsl = qsl_of(NSC - 1)

                def t_oproj_mms(m, ots):
                    op = t_slots[m % 6]
                    for ot in ots:
                        nc.tensor.matmul(
                            op,
                            wo_sb[:, ot, m * 128 : (m + 1) * 128],
                            attnT_q[:, ot, :],
                            start=(ot == 0),
                            stop=(ot == VF // 128 - 1),
                        )

                def t_oproj_fin(m):
                    t_oproj_mms(m, (3,))
                    ob = obs.tile([128, SC], F32, tag="ob", name="ob")
                    # ACT is idle after the last exp: share the tail copies
                    if m % 2 == 0:
                        nc.vector.tensor_copy(ob[:], t_slots[m % 6])
                    else:
                        nc.scalar.copy(ob[:], t_slots[m % 6])
                    nc.sync.dma_start(
                        out=outT[m * 128 : (m + 1) * 128, qsl], in_=ob[:]
                    )

                for m in range(6):
                    t_oproj_mms(m, (0, 1, 2))
                for m in range(6):
                    t_oproj_fin(m)
                for m in (6, 7):
                    t_oproj_mms(m, (0, 1, 2))
                    t_oproj_fin(m)
    _split_excess_waits(nc)
    return nc


def _prep_inputs(cos, sin, hidden_states, w_qkv, w_o):
    """Per-core host-side sharding/transpose/cast. Returns list of in_maps."""
    bf = np.float16
    cos = np.asarray(cos, np.float32)
    sin = np.asarray(sin, np.float32)
    hidden_states = np.asarray(hidden_states, np.float32)
    w_qkv = np.asarray(w_qkv, np.float32)
    w_o = np.asarray(w_o, np.float32)

    cosT = cos.T  # [64, S]
    cos_t = np.ascontiguousarray(np.tile(cosT, (2, 1))).astype(bf)
    # sin multiplier aligned to the *source* partitions of the rot ops:
    # rows [0:32] = +sin[32:64] (multiplies src q[0:32] -> dest [32:64]),
    # rows [32:64] = -sin[0:32] (multiplies src q[32:64] -> dest [0:32]).
    sinT = sin.T
    sin_t = np.ascontiguousarray(
        np.tile(np.concatenate([sinT[32:], -sinT[:32]], 0), (2, 1))
    ).astype(bf)

    in_maps = []
    for core in range(8):
        b, g = core // 2, core % 2
        hT = hidden_states[b].T  # [H, S]
        hT_t = np.ascontiguousarray(
            hT.reshape(HT, 128, S).transpose(1, 0, 2)
        ).astype(bf)
        qs, ks, vs = g * VF, NH * HD + g * VF, 2 * NH * HD + g * VF
        # interleave per pair: [k_p0, q_p0, k_p1, q_p1, ...]
        blocks = []
        for p in range(PAIRS):
            blocks.append(w_qkv[ks + p * 128 : ks + (p + 1) * 128])
            blocks.append(w_qkv[qs + p * 128 : qs + (p + 1) * 128])
        wqk_rows = np.concatenate(blocks, 0)  # [QKF, H]
        wqk_t = np.ascontiguousarray(
            wqk_rows.T.reshape(HT, 128, QKF).transpose(1, 0, 2)
        ).astype(bf)
        wv_t = np.ascontiguousarray(
            w_qkv[vs : vs + VF].T.reshape(HT, 128, VF).transpose(1, 0, 2)
        ).astype(bf)
        woT = w_o[:, g * VF : (g + 1) * VF].T  # [VF, H]
        wo_t = np.ascontiguousarray(
            woT.reshape(VF // 128, 128, H).transpose(1, 0, 2)
        ).astype(bf)
        in_maps.append(
            {
                "hT": hT_t,
                "wqk": wqk_t,
                "wv": wv_t,
                "wo": wo_t,
                "cos_t": cos_t,
                "sin_t": sin_t,
            }
        )
    return in_maps


def kernel(cos, sin, hidden_states, w_qkv, w_o, _trace=False):
    global _CACHED_NC
    if _CACHED_NC is None:
        _CACHED_NC = _build_nc()
    nc = _CACHED_NC
    in_maps = _prep_inputs(cos, sin, hidden_states, w_qkv, w_o)
    res = run_bass_kernel_spmd(nc, in_maps, core_ids=list(range(8)), trace=_trace)
    outs = [r["outT"] for r in res.results]
    out = np.empty((B, S, H), np.float32)
    for b in range(B):
        out[b] = (outs[2 * b] + outs[2 * b + 1]).T
    if _trace:
        return out, res
    return out


# revision 45
# speedup vs baseline: 1.2642x; 1.0095x over previous
"""Trainium2 Bass kernel for nn_Attention: fused QKV + RoPE + softmax attention + o_proj.

Sharding (8 cores): core c -> (batch b = c//2, head-half g = c%2).
Each core computes 8 of 16 heads for one batch:
  - QKV projection (fp16 matmuls, weights pre-transposed/tiled on host)
  - RoPE on DVE (d-on-partition layout, rotation = partition-block swap)
  - scores^T [kpos, q] per head pair, row-packed via tile_position
  - exp on ACT (1/sqrt(d) folded into the activation scale), fp16 out
  - P@V transposed: probs chunk is the *stationary* operand, V (64 dims +
    ones column = 65) is the moving operand -> out [128 q, 65] accumulated
    over kpos; column 64 is the softmax denominator
  - normalize on DVE with a per-partition reciprocal scalar (denominator
    now lives on the q partition), fp16 attn [q, d]
  - attn [q, d] -> [d, q] via DMA XBAR transpose (SBUF->SBUF, no PE/PSUM)
  - o_proj with w_o columns sharded by head; partial out^T [H, S] fp32;
    host sums the two head-half partials per batch
Emission is globally software-pipelined: scores lead exp/P@V by 2; V proj and
the K proj of later pairs stream inside the first q-chunk's exp windows;
o_proj of qc-1 and Q-proj of qc+1 hide under the later exp windows.
"""
import os
import sys

sys.path.insert(0, "/opt/trn_rl_repo")

import numpy as np
import ml_dtypes

import concourse.bass as bass
import concourse.mybir as mybir
import concourse.tile as tile
from concourse import library_config
from concourse.bass_utils import run_bass_kernel_spmd
from concourse.vector_clock import ScopedClock, VectorClock

# ---------------------------------------------------------------------------
# Patch TileContext._drain_and_barrier: the walrus build in this container
# allows only ONE sync-wait per instruction; Tile's tail drain carries one
# wait per active proc.  Split them into single-wait NOPs on SP.
N_PROCS = 27


def _patched_drain_and_barrier(self, tick_clock, wait_clock):
    nc = self.nc
    gc = tick_clock.global_clock
    for p in range(N_PROCS):
        t = gc[p]
        if t > 0:
            nop = nc.sync.nop(nofuse=True)
            vc = VectorClock([t if q == p else 0 for q in range(N_PROCS)])
            wait_clock.add_sem_waits(nop.ins, ScopedClock({None: vc}))
    nc.sync.drain()
    nc.all_engine_barrier()
    assert self.sems is not None
    popped = nc._tile_sem_poison_stack.pop()
    assert popped is self._sem_poison
    nc.clear_and_free_semaphores(list(self.sems.allocated().values()))
    nc.all_engine_barrier()


tile.TileContext._drain_and_barrier = _patched_drain_and_barrier


def _split_excess_waits(nc):
    """walrus in this container accepts 1 sync-wait per instruction (2 on
    EventSemaphore).  Move excess waits onto EventSemaphore instructions
    inserted just before, on the same engine."""
    for f in nc.m.functions:
        for bb in f.blocks:
            new_insts = []
            changed = False
            for ins in bb.instructions:
                si = ins.sync_info
                waits = list(si.on_wait) if si is not None else []
                cap = 2 if isinstance(ins, mybir.InstEventSemaphore) else 1
                if len(waits) > cap:
                    changed = True
                    excess = waits[: len(waits) - cap]
                    for i in range(0, len(excess), 2):
                        ev = mybir.InstEventSemaphore(
                            name=f"I-{nc.next_id()}",
                            engine=ins.engine,
                            ins=[],
                            outs=[],
                            sync_info=mybir.SyncInfo(
                                on_wait=excess[i : i + 2], on_update=[]
                            ),
                        )
                        nc.register_instruction(ev)
                        new_insts.append(ev)
                    si.on_wait = waits[len(waits) - cap :]
                new_insts.append(ins)
            if changed:
                bb.instructions[:] = new_insts
# ---------------------------------------------------------------------------

B, S, H, NH, HD = 4, 2048, 1024, 16, 64
HPC = NH // 2          # heads per core
PAIRS = HPC // 2       # head pairs per core
HT = H // 128          # hidden-dim tiles
QKF = 2 * HPC * HD     # q+k features per core (1024)
VF = HPC * HD          # v features per core (512)
SC = 512               # seq chunk (psum bank)
NSC = S // SC
KT = S // 128          # kpos tiles
QT = SC // 128         # q sub-tiles per chunk
BF = mybir.dt.float16  # fp16: 10-bit mantissa, same PE/DVE speed as bf16
F32 = mybir.dt.float32
EXP_SCALE = 1.0 / float(np.sqrt(HD))

# ---------------------------------------------------------------------------
# Stream order is (pair, qc, kt): each pair runs its four q-chunk windows
# back to back, so the K/V/Q projections for pair p+1 spread across pair p's
# four exp windows instead of all crowding into the first q-chunk.
# Hook schedule per window (pair, qc): kt -> list of work items; kt 12-15 are
# kept hook-free so the DVE normalize and the scores leads at each window
# boundary are never queued behind hook work.
#   ("vo", st): V slice for this pair (own), two iterations ahead of use
#   ("vp", st): V slice prefetch for pair+1
#   ("k", c) / ("qn",) / ("qs", c): K chunk / Q c0 for pair+1, own Q chunk c
#   ("o", m): o_proj m-tile of chunk qc-1 (pair 3 windows only)
_HOOKS = {}
for _qc in range(4):
    for _kt in range(16):
        _HOOKS[(_qc, _kt)] = []
for _kt in range(12):  # own V slices, st 2..13 then 14/15 doubled at 10/11
    _HOOKS[(0, _kt)].append(("vo", _kt + 2))
_HOOKS[(0, 10)].append(("vo", 14))
_HOOKS[(0, 11)].append(("vo", 15))
for _i, _st in enumerate(range(5, 16)):  # prefetch pair+1 V across qc1-3
    _qc = 1 + _i // 4
    _HOOKS[(_qc, (_i % 4) * 2)].append(("vp", _st))
_HOOKS[(1, 3)].append(("k", 0))
_HOOKS[(1, 9)].append(("k", 1))
_HOOKS[(2, 3)].append(("k", 2))
_HOOKS[(2, 9)].append(("k", 3))
_HOOKS[(3, 3)].append(("qn",))
for _qc in range(3):  # own q chunk qc+1
    _HOOKS[(_qc, 5)].append(("qs", _qc + 1))
# o_proj slots in pair-3 windows: first at kt3 so the previous chunk's
# pair-3 XBAR transpose (~2.9us) has landed; none at kt 13-15.
_OPROJ_KTS = (3, 4, 6, 7, 9, 10, 11, 12)
# ---------------------------------------------------------------------------

_CACHED_NC = None


def _build_nc():
    nc = bass.Bass()
    hT = nc.declare_dram_parameter("hT", [128, HT, S], BF, isOutput=False)
    # wqk feature order (host-packed): [k_p0, q_p0, k_p1, q_p1, ...] so the
    # first 256 columns are everything pair 0 needs to start.
    wqk = nc.declare_dram_parameter("wqk", [128, HT, QKF], BF, isOutput=False)
    wv = nc.declare_dram_parameter("wv", [128, HT, VF], BF, isOutput=False)
    wo = nc.declare_dram_parameter("wo", [128, VF // 128, H], BF, isOutput=False)
    cos_t = nc.declare_dram_parameter("cos_t", [128, S], BF, isOutput=False)
    sin_t = nc.declare_dram_parameter("sin_t", [128, S], BF, isOutput=False)
    outT = nc.declare_dram_parameter("outT", [H, S], BF, isOutput=True)

    Exp = mybir.ActivationFunctionType.Exp

    with tile.TileContext(nc) as tc:
        with tc.tile_pool(name="singles", bufs=1) as singles:
            hT_sb = singles.tile([128, HT, S], BF)
            wqk_sb = singles.tile([128, HT, QKF], BF)
            wv_sb = singles.tile([128, HT, VF], BF)
            wo_sb = singles.tile([128, VF // 128, H], BF)
            cos_sb = singles.tile([128, S], BF)
            sin_sb = singles.tile([128, S], BF)
            q_rope = singles.tile([128, PAIRS, S], BF)
            k_rope = singles.tile([128, PAIRS, S], BF)
            vext = singles.tile([128, KT, HPC * 65], BF)
            zeros_sb = singles.tile([128, QT * 65], BF)
            nc.vector.memset(zeros_sb[:], 0.0)

            # DMA priority order: pair-0 weights first, then hidden, rope
            # tables, V weights, remaining QKV weights, o_proj weights.
            # DMA priority: pair-0 q/k weights (one fused transfer), hidden
            # in 4 fragments (the first projection chains track arrival),
            # chunk-0 rope tables, V weights, remaining rope chunks, the
            # rest of the QKV weights, and o_proj weights.  Transfers are
            # fused where the consumer granularity allows: the HWDGE setup
            # (~630ns each) otherwise delays the critical hT stream.
            nc.sync.dma_start(out=wqk_sb[:, :, 0:256], in_=wqk[:, :, 0:256])
            for g in range(4):
                ksl = slice(2 * g, 2 * g + 2)
                nc.sync.dma_start(out=hT_sb[:, ksl, :], in_=hT[:, ksl, :])
            nc.sync.dma_start(out=cos_sb[:, 0:SC], in_=cos_t[:, 0:SC])
            nc.sync.dma_start(out=sin_sb[:, 0:SC], in_=sin_t[:, 0:SC])
            nc.sync.dma_start(out=wv_sb[:], in_=wv[:])
            for c in range(1, NSC):
                csl = slice(c * SC, (c + 1) * SC)
                nc.sync.dma_start(out=cos_sb[:, csl], in_=cos_t[:, csl])
                nc.sync.dma_start(out=sin_sb[:, csl], in_=sin_t[:, csl])
            nc.sync.dma_start(out=wqk_sb[:, :, 256:QKF], in_=wqk[:, :, 256:QKF])
            nc.sync.dma_start(out=wo_sb[:], in_=wo[:])
            nc.gpsimd.memset(vext[:], 1.0)

            # ---- pools (PSUM: sps 4 + pvps 2 + projps 1 + opps 1 = 8) ----
            with (
                tc.tile_pool(name="sps", bufs=2, space="PSUM") as sps,
                tc.tile_pool(name="pvps", bufs=1, space="PSUM") as pvps,
                tc.tile_pool(name="projps", bufs=1, space="PSUM") as projps,
                tc.tile_pool(name="opps", bufs=1, space="PSUM") as opps,
                tc.tile_pool(name="raws", bufs=3) as raws,
                tc.tile_pool(name="ropet", bufs=3) as ropet,
                tc.tile_pool(name="eps", bufs=6) as eps,
                tc.tile_pool(name="recs", bufs=4) as recs,
                tc.tile_pool(name="attns", bufs=4) as attns,
                tc.tile_pool(name="attnTs", bufs=4) as attnTs,
                tc.tile_pool(name="obs", bufs=4) as obs,
                tc.tile_pool(name="obg", bufs=2) as obg,
            ):

                def alloc_ps(pool):
                    """[128, SC] f32 psum AP from pool, one tag per pool so
                    every pool stays single-slot (1 bank; sps slots 2 banks)."""
                    if pool is sps:
                        s2t = sps.tile([128, 2, SC], F32, tag="s2", name="s2t")
                        return s2t[:, 0, :]
                    if pool is projps:
                        return projps.tile([128, SC], F32, tag="pj", name="pj")[:]
                    return opps.tile([128, SC], F32, tag="op", name="op")[:]

                def rope_apply(raw, m, c, off, ln):
                    """RoPE on columns [off, off+ln) of chunk c of feature
                    tile m, from the fp16 raw tile into q_rope/k_rope."""
                    pair = m // 2
                    dst_t = k_rope if m % 2 == 0 else q_rope
                    lo, hi = c * SC + off, c * SC + off + ln
                    cs = cos_sb[:, lo:hi]
                    sn = sin_sb[:, lo:hi]
                    dst = dst_t[:, pair, lo:hi]
                    r = raw[:, off : off + ln]
                    t1 = ropet.tile([128, SC], BF, tag="t1")
                    t2 = ropet.tile([128, SC], BF, tag="t2")
                    t1 = t1[:, 0:ln]
                    t2 = t2[:, 0:ln]
                    nc.vector.tensor_mul(t1, r, cs)
                    nc.vector.tensor_mul(t2[0:32], r[32:64], sn[32:64])
                    nc.vector.tensor_mul(t2[32:64], r[0:32], sn[0:32])
                    nc.vector.tensor_mul(t2[64:96], r[96:128], sn[96:128])
                    nc.vector.tensor_mul(t2[96:128], r[64:96], sn[64:96])
                    nc.vector.tensor_add(dst, t1, t2)

                def proj_chunk(m, c, psum_pool=None, copy_eng="vector"):
                    """Project q/k feature tile m (pair m//2, k if m even else
                    q) for seq chunk c, apply RoPE."""
                    ps = alloc_ps(sps if psum_pool is None else psum_pool)
                    for k in range(HT):
                        nc.tensor.matmul(
                            ps,
                            wqk_sb[:, k, m * 128 : (m + 1) * 128],
                            hT_sb[:, k, c * SC : (c + 1) * SC],
                            start=(k == 0),
                            stop=(k == HT - 1),
                        )
                    raw = raws.tile([128, SC], BF)
                    if copy_eng == "vector":
                        nc.vector.tensor_copy(raw[:], ps)
                    else:
                        nc.scalar.copy(raw[:], ps)
                    rope_apply(raw, m, c, 0, SC)

                def v_proj(st, pair, psum_pool, copy_eng="vector"):
                    """V slice for one head pair (128 features) of kpos tile
                    st; cheap (128 moving cols) so it never bursts the PE."""
                    ps = alloc_ps(psum_pool)
                    fsl = slice(pair * 128, (pair + 1) * 128)
                    for k in range(HT):
                        nc.tensor.matmul(
                            ps[:, 0:128],
                            hT_sb[:, k, st * 128 : (st + 1) * 128],
                            wv_sb[:, k, fsl],
                            start=(k == 0),
                            stop=(k == HT - 1),
                        )
                    vdst = vext[:, st, :].rearrange("p (h x) -> p h x", x=65)[
                        :, 2 * pair : 2 * pair + 2, 0:64
                    ]
                    vsrc = ps[:, 0:128].rearrange("p (h x) -> p h x", x=64)
                    if copy_eng == "vector":
                        nc.vector.tensor_copy(vdst, vsrc)
                    else:
                        nc.scalar.copy(vdst, vsrc)

                def emit_scores(pair, kt, qsl):
                    ksl = slice(kt * 128, (kt + 1) * 128)
                    s2 = sps.tile([128, 2, SC], F32, tag="s2", name="s2")
                    nc.tensor.matmul(
                        s2[:, 0, :],
                        k_rope[0:64, pair, ksl],
                        q_rope[0:64, pair, qsl],
                        start=True,
                        stop=True,
                        tile_position=(0, 0),
                    )
                    nc.tensor.matmul(
                        s2[:, 1, :],
                        k_rope[64:128, pair, ksl],
                        q_rope[64:128, pair, qsl],
                        start=True,
                        stop=True,
                        tile_position=(64, 0),
                    )
                    return s2

                def o_proj_m(qc, m, attnT_q, pool):
                    qsl = slice(qc * SC, (qc + 1) * SC)
                    op = alloc_ps(pool)
                    for ot in range(VF // 128):
                        nc.tensor.matmul(
                            op,
                            wo_sb[:, ot, m * 128 : (m + 1) * 128],
                            attnT_q[:, ot, :],
                            start=(ot == 0),
                            stop=(ot == VF // 128 - 1),
                        )
                    ob = obs.tile([128, SC], BF, tag="ob", name="ob")
                    nc.vector.tensor_copy(ob[:], op)
                    nc.sync.dma_start(out=outT[m * 128 : (m + 1) * 128, qsl], in_=ob[:])

                # ---- head phase.  The first exp is gated by k-c0/q-c0 of
                # pair 0: run both projection chains k-major (so they track
                # the hT fragment arrivals), copy on ACT (idle until the
                # first exp), rope q whole but k in 128-col slices so the
                # first scores only wait on the first slice.
                hg = sps.tile([128, 2, SC], F32, tag="s2", name="hg")
                for k in range(HT):
                    for j in range(2):
                        nc.tensor.matmul(
                            hg[:, j, :],
                            wqk_sb[:, k, j * 128 : (j + 1) * 128],
                            hT_sb[:, k, 0:SC],
                            start=(k == 0),
                            stop=(k == HT - 1),
                        )
                raw_k = raws.tile([128, SC], BF)
                raw_q = raws.tile([128, SC], BF)
                nc.scalar.copy(raw_q[:], hg[:, 1, :])
                nc.scalar.copy(raw_k[:], hg[:, 0, :])
                rope_apply(raw_q, 1, 0, 0, SC)               # q pair0 c0
                for sl in range(QT):
                    rope_apply(raw_k, 0, 0, sl * 128, 128)   # k pair0 c0
                # own V st0/1 plus the first pair-1 V prefetches ride the
                # sps banks and the head's idle PE while DMAs stream
                for vstart, vpair in ((0, 0), (1, 0), (0, 1), (1, 1), (2, 1),
                                      (3, 1), (4, 1)):
                    v_proj(vstart, vpair, sps, copy_eng="scalar")

                # ---- globally software-pipelined attention stream ----
                flat = [
                    (p, qc, k)
                    for p in range(PAIRS)
                    for qc in range(NSC)
                    for k in range(KT)
                ]

                def qsl_of(qc):
                    return slice(qc * SC, (qc + 1) * SC)

                hookn = [0]

                def hook(pair, qc, kt, attnT_tiles):
                    """PE/DMA producer work interleaved into iteration
                    (pair, qc, kt), emitted before the scores lead.  Pool
                    choice alternates globally so consecutive hook items
                    never reuse the same psum bank back to back."""
                    def next_pool():
                        pool = opps if hookn[0] % 2 == 0 else projps
                        hookn[0] += 1
                        return pool

                    for item in _HOOKS[(qc, kt)]:
                        kind = item[0]
                        if kind == "vo":
                            if pair == 0:
                                v_proj(item[1], 0, next_pool())
                        elif kind == "vp":
                            if pair < PAIRS - 1:
                                v_proj(item[1], pair + 1, next_pool())
                        elif kind == "k":
                            if pair < PAIRS - 1:
                                proj_chunk(2 * (pair + 1), item[1], next_pool())
                        elif kind == "qn":
                            if pair < PAIRS - 1:
                                proj_chunk(2 * (pair + 1) + 1, 0, next_pool())
                        elif kind == "qs":
                            # pair 3's own q chunks are prefetched from the
                            # pair-1/2 groups so its o_proj windows stay light
                            if pair < PAIRS - 1:
                                proj_chunk(2 * pair + 1, item[1], next_pool())
                    if pair == 1 and kt == 1 and qc in (1, 2):
                        proj_chunk(7, qc, next_pool())       # q pair3 c1/c2
                    if pair == 2 and kt == 1 and qc == 1:
                        proj_chunk(7, 3, next_pool())        # q pair3 c3
                    # pairs 2/3 did not get V st0-4 from the head (only the
                    # pair-1 prefetch rides there): produce them here
                    if pair in (1, 2) and qc >= 1 and kt in (8, 10):
                        st = {(1, 8): 0, (1, 10): 1, (2, 8): 2, (2, 10): 3,
                              (3, 8): 4}.get((qc, kt))
                        if st is not None:
                            v_proj(st, pair + 1, next_pool())
                    if pair == PAIRS - 1 and qc >= 1 and kt in _OPROJ_KTS:
                        m = _OPROJ_KTS.index(kt)
                        o_proj_m(qc - 1, m, attnT_tiles[qc - 1], next_pool())

                def pv_mms(pair, qc, kt, e, heads):
                    # acc banks hold four 65-col accumulation groups each; a
                    # start=True would zero the whole bank and wipe sibling
                    # groups, so the accs are memset-zeroed at kt0 and every
                    # matmul is a pure accumulate.
                    accA, accB = accs[(pair, qc)]
                    vx = vext[:, kt, :].rearrange("p (h x) -> p h x", x=65)
                    for h in heads:
                        acc = accA if h == 0 else accB
                        for qs in range(QT):
                            nc.tensor.matmul(
                                acc[:, qs, :],
                                e[:, h, qs * 128 : (qs + 1) * 128],
                                vx[:, 2 * pair + h, :],
                                start=False,
                                stop=(kt == KT - 1),
                                skip_group_check=True,
                            )

                s2q = {
                    0: emit_scores(flat[0][0], flat[0][2], qsl_of(flat[0][1])),
                    1: emit_scores(flat[1][0], flat[1][2], qsl_of(flat[1][1])),
                }
                # k pair0 c1-3 after the scores prestage, via the proj pools
                # so they do not recycle the prestaged scores banks
                proj_chunk(0, 1, projps, copy_eng="scalar")
                proj_chunk(0, 2, opps, copy_eng="scalar")
                proj_chunk(0, 3, projps, copy_eng="scalar")
                accs = {}
                attn_tiles = {}
                attnT_tiles = {}
                ekt = {}
                for i, (pair, qc, kt) in enumerate(flat):
                    if pair == 0 and kt == 0:
                        attn_tiles[qc] = attns.tile(
                            [128, PAIRS, QT, 128], BF, tag="attn", name="attn"
                        )
                    if kt == 0:
                        acAt = pvps.tile([128, SC], F32, tag="acA", name="acAt")
                        acBt = pvps.tile([128, SC], F32, tag="acB", name="acBt")
                        accA = acAt[:, 0 : QT * 65].rearrange(
                            "p (q x) -> p q x", x=65
                        )
                        accB = acBt[:, 0 : QT * 65].rearrange(
                            "p (q x) -> p q x", x=65
                        )
                        accs[(pair, qc)] = (accA, accB)
                        nc.vector.memset(accA, 0.0)
                        nc.vector.memset(accB, 0.0)
                    hook(pair, qc, kt, attnT_tiles)
                    s2 = s2q.pop(i)
                    e = eps.tile([128, 2, SC], BF)
                    nc.scalar.activation(out=e[:], in_=s2[:], func=Exp, scale=EXP_SCALE)
                    if i + 2 < len(flat):
                        npair, nqc, nkt = flat[i + 2]
                        s2q[i + 2] = emit_scores(npair, nkt, qsl_of(nqc))
                    # P@V lags exp: head A by one iteration, head B by two.
                    # kt0's P@V waits on the previous window's normalize (psum
                    # WAR); the lag absorbs that latency and keeps any parked
                    # group within the 4-deep PE wait queue.
                    ekt[kt] = e
                    if kt >= 2:
                        pv_mms(pair, qc, kt - 2, ekt[kt - 2], (0,))
                    if kt >= 3:
                        pv_mms(pair, qc, kt - 3, ekt[kt - 3], (1,))
                    if kt == KT - 1:
                        pv_mms(pair, qc, KT - 2, ekt[KT - 2], (0,))
                        pv_mms(pair, qc, KT - 1, ekt[KT - 1], (0,))
                        pv_mms(pair, qc, KT - 3, ekt[KT - 3], (1,))
                        pv_mms(pair, qc, KT - 2, ekt[KT - 2], (1,))
                        pv_mms(pair, qc, KT - 1, ekt[KT - 1], (1,))
                        rec = recs.tile([128, 2, QT], F32)
                        accA, accB = accs[(pair, qc)]
                        at = attn_tiles[qc]
                        nc.vector.reciprocal(rec[:, 0, :], accA[:, :, 64])
                        nc.vector.reciprocal(rec[:, 1, :], accB[:, :, 64])
                        for h, acc in ((0, accA), (1, accB)):
                            for qs in range(QT):
                                nc.vector.tensor_scalar_mul(
                                    at[:, pair, qs, h * 64 : (h + 1) * 64],
                                    acc[:, qs, 0:64],
                                    rec[:, h, qs : qs + 1],
                                )
                        # transpose this pair's attn [q, d] -> [d, q] in ONE
                        # XBAR DMA (no PE/PSUM); attnT(qc) complete once the
                        # pair-3 transpose lands
                        if pair == 0:
                            attnT_tiles[qc] = attnTs.tile(
                                [128, PAIRS, SC], BF, tag="aT", name="aT"
                            )
                        nc.sync.dma_start_transpose(
                            out=attnT_tiles[qc][:, pair, :].rearrange(
                                "p (di m) -> p di m", m=128
                            ),
                            in_=at[:, pair, :, :],
                        )

                # ---- tail: last chunk's o_proj.  Six psum slots (opps,
                # projps, and the now-idle scores banks) let the ot 0-2
                # matmuls prefill while the pair-3 transpose is in flight;
                # the ot-3 matmuls and copies then stream without bank waits.
                attnT_q = attnT_tiles[NSC - 1]
                t_s2a = sps.tile([128, 2, SC], F32, tag="s2", name="t_s2a")
                t_s2b = sps.tile([128, 2, SC], F32, tag="s2", name="t_s2b")
                t_pva = pvps.tile([128, SC], F32, tag="acA", name="t_pva")
                t_pvb = pvps.tile([128, SC], F32, tag="acB", name="t_pvb")
                t_slots = [
                    alloc_ps(opps),
                    alloc_ps(projps),
                    t_s2a[:, 0, :],
                    t_s2a[:, 1, :],
                    t_s2b[:, 0, :],
                    t_s2b[:, 1, :],
                    t_pva[:],
                    t_pvb[:],
                ]
                qsl = qsl_of(NSC - 1)

                def t_oproj_mms(m, ots):
                    op = t_slots[m]
                    for ot in ots:
                        nc.tensor.matmul(
                            op,
                            wo_sb[:, ot, m * 128 : (m + 1) * 128],
                            attnT_q[:, ot, :],
                            start=(ot == 0),
                            stop=(ot == VF // 128 - 1),
                        )

                def t_oproj_fin(m, grp):
                    t_oproj_mms(m, (3,))
                    # ACT is idle after the last exp: share the tail copies
                    if m % 2 == 0:
                        nc.vector.tensor_copy(grp[:, m % 4, :], t_slots[m])
                    else:
                        nc.scalar.copy(grp[:, m % 4, :], t_slots[m])

                for m in range(HT):
                    t_oproj_mms(m, (0, 1, 2))
                for g in range(2):
                    grp = obg.tile([128, 4, SC], BF, tag="og", name="og")
                    for m in range(4 * g, 4 * g + 4):
                        t_oproj_fin(m, grp)
                    # one batched DMA per 4 output tiles: the tail is HWDGE
                    # setup-latency bound, not bandwidth bound
                    nc.sync.dma_start(
                        out=outT[g * SC : (g + 1) * SC, qsl].rearrange(
                            "(m p) s -> p m s", p=128
                        ),
                        in_=grp[:],
                    )
    _split_excess_waits(nc)
    return nc


def _prep_inputs(cos, sin, hidden_states, w_qkv, w_o):
    """Per-core host-side sharding/transpose/cast. Returns list of in_maps."""
    bf = np.float16
    cos = np.asarray(cos, np.float32)
    sin = np.asarray(sin, np.float32)
    hidden_states = np.asarray(hidden_states, np.float32)
    w_qkv = np.asarray(w_qkv, np.float32)
    w_o = np.asarray(w_o, np.float32)

    cosT = cos.T  # [64, S]
    cos_t = np.ascontiguousarray(np.tile(cosT, (2, 1))).astype(bf)
    # sin multiplier aligned to the *source* partitions of the rot ops:
    # rows [0:32] = +sin[32:64] (multiplies src q[0:32] -> dest [32:64]),
    # rows [32:64] = -sin[0:32] (multiplies src q[32:64] -> dest [0:32]).
    sinT = sin.T
    sin_t = np.ascontiguousarray(
        np.tile(np.concatenate([sinT[32:], -sinT[:32]], 0), (2, 1))
    ).astype(bf)

    in_maps = []
    for core in range(8):
        b, g = core // 2, core % 2
        hT = hidden_states[b].T  # [H, S]
        hT_t = np.ascontiguousarray(
            hT.reshape(HT, 128, S).transpose(1, 0, 2)
        ).astype(bf)
        qs, ks, vs = g * VF, NH * HD + g * VF, 2 * NH * HD + g * VF
        # interleave per pair: [k_p0, q_p0, k_p1, q_p1, ...]
        blocks = []
        for p in range(PAIRS):
            blocks.append(w_qkv[ks + p * 128 : ks + (p + 1) * 128])
            blocks.append(w_qkv[qs + p * 128 : qs + (p + 1) * 128])
        wqk_rows = np.concatenate(blocks, 0)  # [QKF, H]
        wqk_t = np.ascontiguousarray(
            wqk_rows.T.reshape(HT, 128, QKF).transpose(1, 0, 2)
        ).astype(bf)
        wv_t = np.ascontiguousarray(
            w_qkv[vs : vs + VF].T.reshape(HT, 128, VF).transpose(1, 0, 2)
        ).astype(bf)
        woT = w_o[:, g * VF : (g + 1) * VF].T  # [VF, H]
        wo_t = np.ascontiguousarray(
            woT.reshape(VF // 128, 128, H).transpose(1, 0, 2)
        ).astype(bf)
        in_maps.append(
            {
                "hT": hT_t,
                "wqk": wqk_t,
                "wv": wv_t,
                "wo": wo_t,
                "cos_t": cos_t,
                "sin_t": sin_t,
            }
        )
    return in_maps


def kernel(cos, sin, hidden_states, w_qkv, w_o, _trace=False):
    global _CACHED_NC
    if _CACHED_NC is None:
        _CACHED_NC = _build_nc()
    nc = _CACHED_NC
    in_maps = _prep_inputs(cos, sin, hidden_states, w_qkv, w_o)
    res = run_bass_kernel_spmd(nc, in_maps, core_ids=list(range(8)), trace=_trace)
    outs = [r["outT"] for r in res.results]
    out = np.empty((B, S, H), np.float32)
    for b in range(B):
        out[b] = (
            outs[2 * b].astype(np.float32) + outs[2 * b + 1].astype(np.float32)
        ).T
    if _trace:
        return out, res
    return out


# revision 51
# speedup vs baseline: 1.2699x; 1.0045x over previous
"""Trainium2 Bass kernel for nn_Attention: fused QKV + RoPE + softmax attention + o_proj.

Sharding (8 cores): core c -> (batch b = c//2, head-half g = c%2).
Each core computes 8 of 16 heads for one batch:
  - QKV projection (fp16 matmuls, weights pre-transposed/tiled on host)
  - RoPE on DVE (d-on-partition layout, rotation = partition-block swap)
  - scores^T [kpos, q] per head pair, row-packed via tile_position
  - exp on ACT (1/sqrt(d) folded into the activation scale), fp16 out
  - P@V transposed: probs chunk is the *stationary* operand, V (64 dims +
    ones column = 65) is the moving operand -> out [128 q, 65] accumulated
    over kpos; column 64 is the softmax denominator
  - normalize on DVE with a per-partition reciprocal scalar (denominator
    now lives on the q partition), fp16 attn [q, d]
  - attn [q, d] -> [d, q] via DMA XBAR transpose (SBUF->SBUF, no PE/PSUM)
  - o_proj with w_o columns sharded by head; partial out^T [H, S] fp32;
    host sums the two head-half partials per batch
Emission is globally software-pipelined: scores lead exp/P@V by 2; V proj and
the K proj of later pairs stream inside the first q-chunk's exp windows;
o_proj of qc-1 and Q-proj of qc+1 hide under the later exp windows.
"""
import os
import sys

sys.path.insert(0, "/opt/trn_rl_repo")

import numpy as np
import ml_dtypes

import concourse.bass as bass
import concourse.mybir as mybir
import concourse.tile as tile
from concourse import library_config
from concourse.bass_utils import run_bass_kernel_spmd
from concourse.vector_clock import ScopedClock, VectorClock

# ---------------------------------------------------------------------------
# Patch TileContext._drain_and_barrier: the walrus build in this container
# allows only ONE sync-wait per instruction; Tile's tail drain carries one
# wait per active proc.  Split them into single-wait NOPs on SP.
N_PROCS = 27


def _patched_drain_and_barrier(self, tick_clock, wait_clock):
    nc = self.nc
    gc = tick_clock.global_clock
    for p in range(N_PROCS):
        t = gc[p]
        if t > 0:
            nop = nc.sync.nop(nofuse=True)
            vc = VectorClock([t if q == p else 0 for q in range(N_PROCS)])
            wait_clock.add_sem_waits(nop.ins, ScopedClock({None: vc}))
    nc.sync.drain()
    nc.all_engine_barrier()
    assert self.sems is not None
    popped = nc._tile_sem_poison_stack.pop()
    assert popped is self._sem_poison
    nc.clear_and_free_semaphores(list(self.sems.allocated().values()))
    nc.all_engine_barrier()


tile.TileContext._drain_and_barrier = _patched_drain_and_barrier


def _split_excess_waits(nc):
    """walrus in this container accepts 1 sync-wait per instruction (2 on
    EventSemaphore).  Move excess waits onto EventSemaphore instructions
    inserted just before, on the same engine."""
    for f in nc.m.functions:
        for bb in f.blocks:
            new_insts = []
            changed = False
            for ins in bb.instructions:
                si = ins.sync_info
                waits = list(si.on_wait) if si is not None else []
                cap = 2 if isinstance(ins, mybir.InstEventSemaphore) else 1
                if len(waits) > cap:
                    changed = True
                    excess = waits[: len(waits) - cap]
                    for i in range(0, len(excess), 2):
                        ev = mybir.InstEventSemaphore(
                            name=f"I-{nc.next_id()}",
                            engine=ins.engine,
                            ins=[],
                            outs=[],
                            sync_info=mybir.SyncInfo(
                                on_wait=excess[i : i + 2], on_update=[]
                            ),
                        )
                        nc.register_instruction(ev)
                        new_insts.append(ev)
                    si.on_wait = waits[len(waits) - cap :]
                new_insts.append(ins)
            if changed:
                bb.instructions[:] = new_insts
# ---------------------------------------------------------------------------

B, S, H, NH, HD = 4, 2048, 1024, 16, 64
HPC = NH // 2          # heads per core
PAIRS = HPC // 2       # head pairs per core
HT = H // 128          # hidden-dim tiles
QKF = 2 * HPC * HD     # q+k features per core (1024)
VF = HPC * HD          # v features per core (512)
SC = 512               # seq chunk (psum bank)
NSC = S // SC
KT = S // 128          # kpos tiles
QT = SC // 128         # q sub-tiles per chunk
BF = mybir.dt.float16  # fp16: 10-bit mantissa, same PE/DVE speed as bf16
F32 = mybir.dt.float32
EXP_SCALE = 1.0 / float(np.sqrt(HD))

# ---------------------------------------------------------------------------
# Stream order is (pair, qc, kt): each pair runs its four q-chunk windows
# back to back, so the K/V/Q projections for pair p+1 spread across pair p's
# four exp windows instead of all crowding into the first q-chunk.
# Hook schedule per window (pair, qc): kt -> list of work items; kt 12-15 are
# kept hook-free so the DVE normalize and the scores leads at each window
# boundary are never queued behind hook work.
#   ("vo", st): V slice for this pair (own), two iterations ahead of use
#   ("vp", st): V slice prefetch for pair+1
#   ("k", c) / ("qn",) / ("qs", c): K chunk / Q c0 for pair+1, own Q chunk c
#   ("o", m): o_proj m-tile of chunk qc-1 (pair 3 windows only)
_HOOKS = {}
for _qc in range(4):
    for _kt in range(16):
        _HOOKS[(_qc, _kt)] = []
for _kt in range(12):  # own V slices, st 2..13 then 14/15 doubled at 10/11
    _HOOKS[(0, _kt)].append(("vo", _kt + 2))
_HOOKS[(0, 10)].append(("vo", 14))
_HOOKS[(0, 11)].append(("vo", 15))
for _i, _st in enumerate(range(5, 16)):  # prefetch pair+1 V across qc1-3
    _qc = 1 + _i // 4
    _HOOKS[(_qc, (_i % 4) * 2)].append(("vp", _st))
_HOOKS[(1, 3)].append(("k", 0))
_HOOKS[(1, 9)].append(("k", 1))
_HOOKS[(2, 3)].append(("k", 2))
_HOOKS[(2, 9)].append(("k", 3))
_HOOKS[(3, 3)].append(("qn",))
for _qc in range(3):  # own q chunk qc+1
    _HOOKS[(_qc, 5)].append(("qs", _qc + 1))
# o_proj slots in pair-3 windows: first at kt3 so the previous chunk's
# pair-3 XBAR transpose (~2.9us) has landed; none at kt 13-15.
_OPROJ_KTS = (3, 4, 6, 7, 9, 10, 11, 12)
# ---------------------------------------------------------------------------

_CACHED_NC = None


def _build_nc():
    nc = bass.Bass()
    hT = nc.declare_dram_parameter("hT", [128, HT, S], BF, isOutput=False)
    # wqk feature order (host-packed): [k_p0, q_p0, k_p1, q_p1, ...] so the
    # first 256 columns are everything pair 0 needs to start.
    wqk = nc.declare_dram_parameter("wqk", [128, HT, QKF], BF, isOutput=False)
    wv = nc.declare_dram_parameter("wv", [128, HT, VF], BF, isOutput=False)
    wo = nc.declare_dram_parameter("wo", [128, VF // 128, H], BF, isOutput=False)
    cos_t = nc.declare_dram_parameter("cos_t", [128, S], BF, isOutput=False)
    sin_t = nc.declare_dram_parameter("sin_t", [128, S], BF, isOutput=False)
    outT = nc.declare_dram_parameter("outT", [H, S], BF, isOutput=True)

    Exp = mybir.ActivationFunctionType.Exp

    with tile.TileContext(nc) as tc:
        with tc.tile_pool(name="singles", bufs=1) as singles:
            hT_sb = singles.tile([128, HT, S], BF)
            wqk_sb = singles.tile([128, HT, QKF], BF)
            wv_sb = singles.tile([128, HT, VF], BF)
            wo_sb = singles.tile([128, VF // 128, H], BF)
            cos_sb = singles.tile([128, S], BF)
            sin_sb = singles.tile([128, S], BF)
            q_rope = singles.tile([128, PAIRS, S], BF)
            k_rope = singles.tile([128, PAIRS, S], BF)
            vext = singles.tile([128, KT, HPC * 65], BF)
            zeros_sb = singles.tile([128, QT * 65], BF)
            nc.vector.memset(zeros_sb[:], 0.0)

            # DMA priority order: pair-0 weights first, then hidden, rope
            # tables, V weights, remaining QKV weights, o_proj weights.
            # DMA priority: pair-0 q/k weights (one fused transfer), hidden
            # in 4 fragments (the first projection chains track arrival),
            # chunk-0 rope tables, V weights, remaining rope chunks, the
            # rest of the QKV weights, and o_proj weights.  Transfers are
            # fused where the consumer granularity allows: the HWDGE setup
            # (~630ns each) otherwise delays the critical hT stream.
            nc.sync.dma_start(out=wqk_sb[:, :, 0:256], in_=wqk[:, :, 0:256])
            for g in range(4):
                ksl = slice(2 * g, 2 * g + 2)
                nc.sync.dma_start(out=hT_sb[:, ksl, :], in_=hT[:, ksl, :])
            nc.sync.dma_start(out=cos_sb[:, 0:SC], in_=cos_t[:, 0:SC])
            nc.sync.dma_start(out=sin_sb[:, 0:SC], in_=sin_t[:, 0:SC])
            nc.sync.dma_start(out=wv_sb[:], in_=wv[:])
            for c in range(1, NSC):
                csl = slice(c * SC, (c + 1) * SC)
                nc.sync.dma_start(out=cos_sb[:, csl], in_=cos_t[:, csl])
                nc.sync.dma_start(out=sin_sb[:, csl], in_=sin_t[:, csl])
            nc.sync.dma_start(out=wqk_sb[:, :, 256:QKF], in_=wqk[:, :, 256:QKF])
            nc.sync.dma_start(out=wo_sb[:], in_=wo[:])
            nc.gpsimd.memset(vext[:], 1.0)

            # ---- pools (PSUM: sps 4 + pvps 2 + projps 1 + opps 1 = 8) ----
            with (
                tc.tile_pool(name="sps", bufs=2, space="PSUM") as sps,
                tc.tile_pool(name="pvps", bufs=1, space="PSUM") as pvps,
                tc.tile_pool(name="projps", bufs=1, space="PSUM") as projps,
                tc.tile_pool(name="opps", bufs=1, space="PSUM") as opps,
                tc.tile_pool(name="raws", bufs=4) as raws,
                tc.tile_pool(name="ropet", bufs=4) as ropet,
                tc.tile_pool(name="eps", bufs=8) as eps,
                tc.tile_pool(name="recs", bufs=4) as recs,
                tc.tile_pool(name="attns", bufs=4) as attns,
                tc.tile_pool(name="attnTs", bufs=4) as attnTs,
                tc.tile_pool(name="obs", bufs=6) as obs,
                tc.tile_pool(name="obg", bufs=2) as obg,
            ):

                def alloc_ps(pool):
                    """[128, SC] f32 psum AP from pool, one tag per pool so
                    every pool stays single-slot (1 bank; sps slots 2 banks)."""
                    if pool is sps:
                        s2t = sps.tile([128, 2, SC], F32, tag="s2", name="s2t")
                        return s2t[:, 0, :]
                    if pool is projps:
                        return projps.tile([128, SC], F32, tag="pj", name="pj")[:]
                    return opps.tile([128, SC], F32, tag="op", name="op")[:]

                def rope_apply(raw, m, c, off, ln):
                    """RoPE on columns [off, off+ln) of chunk c of feature
                    tile m, from the fp16 raw tile into q_rope/k_rope."""
                    pair = m // 2
                    dst_t = k_rope if m % 2 == 0 else q_rope
                    lo, hi = c * SC + off, c * SC + off + ln
                    cs = cos_sb[:, lo:hi]
                    sn = sin_sb[:, lo:hi]
                    dst = dst_t[:, pair, lo:hi]
                    r = raw[:, off : off + ln]
                    t1 = ropet.tile([128, SC], BF, tag="t1")
                    t2 = ropet.tile([128, SC], BF, tag="t2")
                    t1 = t1[:, 0:ln]
                    t2 = t2[:, 0:ln]
                    nc.vector.tensor_mul(t1, r, cs)
                    nc.vector.tensor_mul(t2[0:32], r[32:64], sn[32:64])
                    nc.vector.tensor_mul(t2[32:64], r[0:32], sn[0:32])
                    nc.vector.tensor_mul(t2[64:96], r[96:128], sn[96:128])
                    nc.vector.tensor_mul(t2[96:128], r[64:96], sn[64:96])
                    nc.vector.tensor_add(dst, t1, t2)

                def proj_chunk(m, c, psum_pool=None, copy_eng="vector"):
                    """Project q/k feature tile m (pair m//2, k if m even else
                    q) for seq chunk c, apply RoPE."""
                    ps = alloc_ps(sps if psum_pool is None else psum_pool)
                    for k in range(HT):
                        nc.tensor.matmul(
                            ps,
                            wqk_sb[:, k, m * 128 : (m + 1) * 128],
                            hT_sb[:, k, c * SC : (c + 1) * SC],
                            start=(k == 0),
                            stop=(k == HT - 1),
                        )
                    raw = raws.tile([128, SC], BF)
                    if copy_eng == "vector":
                        nc.vector.tensor_copy(raw[:], ps)
                    else:
                        nc.scalar.copy(raw[:], ps)
                    rope_apply(raw, m, c, 0, SC)

                def v_proj(st, pair, psum_pool, copy_eng="vector"):
                    """V slice for one head pair (128 features) of kpos tile
                    st; cheap (128 moving cols) so it never bursts the PE."""
                    ps = alloc_ps(psum_pool)
                    fsl = slice(pair * 128, (pair + 1) * 128)
                    for k in range(HT):
                        nc.tensor.matmul(
                            ps[:, 0:128],
                            hT_sb[:, k, st * 128 : (st + 1) * 128],
                            wv_sb[:, k, fsl],
                            start=(k == 0),
                            stop=(k == HT - 1),
                        )
                    vdst = vext[:, st, :].rearrange("p (h x) -> p h x", x=65)[
                        :, 2 * pair : 2 * pair + 2, 0:64
                    ]
                    vsrc = ps[:, 0:128].rearrange("p (h x) -> p h x", x=64)
                    if copy_eng == "vector":
                        nc.vector.tensor_copy(vdst, vsrc)
                    else:
                        nc.scalar.copy(vdst, vsrc)

                def emit_scores(pair, kt, qsl):
                    ksl = slice(kt * 128, (kt + 1) * 128)
                    s2 = sps.tile([128, 2, SC], F32, tag="s2", name="s2")
                    nc.tensor.matmul(
                        s2[:, 0, :],
                        k_rope[0:64, pair, ksl],
                        q_rope[0:64, pair, qsl],
                        start=True,
                        stop=True,
                        tile_position=(0, 0),
                    )
                    nc.tensor.matmul(
                        s2[:, 1, :],
                        k_rope[64:128, pair, ksl],
                        q_rope[64:128, pair, qsl],
                        start=True,
                        stop=True,
                        tile_position=(64, 0),
                    )
                    return s2

                def o_proj_m(qc, m, attnT_q, pool):
                    qsl = slice(qc * SC, (qc + 1) * SC)
                    op = alloc_ps(pool)
                    for ot in range(VF // 128):
                        nc.tensor.matmul(
                            op,
                            wo_sb[:, ot, m * 128 : (m + 1) * 128],
                            attnT_q[:, ot, :],
                            start=(ot == 0),
                            stop=(ot == VF // 128 - 1),
                        )
                    ob = obs.tile([128, SC], BF, tag="ob", name="ob")
                    nc.vector.tensor_copy(ob[:], op)
                    nc.sync.dma_start(out=outT[m * 128 : (m + 1) * 128, qsl], in_=ob[:])

                # ---- head phase.  The first exp is gated by k-c0/q-c0 of
                # pair 0: run both projection chains k-major (so they track
                # the hT fragment arrivals), copy on ACT (idle until the
                # first exp), rope q whole but k in 128-col slices so the
                # first scores only wait on the first slice.
                hg = sps.tile([128, 2, SC], F32, tag="s2", name="hg")
                for k in range(HT):
                    for j in range(2):
                        nc.tensor.matmul(
                            hg[:, j, :],
                            wqk_sb[:, k, j * 128 : (j + 1) * 128],
                            hT_sb[:, k, 0:SC],
                            start=(k == 0),
                            stop=(k == HT - 1),
                        )
                raw_k = raws.tile([128, SC], BF)
                raw_q = raws.tile([128, SC], BF)
                nc.scalar.copy(raw_q[:], hg[:, 1, :])
                nc.scalar.copy(raw_k[:], hg[:, 0, :])
                rope_apply(raw_q, 1, 0, 0, SC)               # q pair0 c0
                for sl in range(QT):
                    rope_apply(raw_k, 0, 0, sl * 128, 128)   # k pair0 c0
                # own V st0/1 plus the first pair-1 V prefetches ride the
                # sps banks and the head's idle PE while DMAs stream
                for vstart, vpair in ((0, 0), (1, 0), (0, 1), (1, 1), (2, 1),
                                      (3, 1), (4, 1)):
                    v_proj(vstart, vpair, sps, copy_eng="scalar")

                # ---- globally software-pipelined attention stream ----
                flat = [
                    (p, qc, k)
                    for p in range(PAIRS)
                    for qc in range(NSC)
                    for k in range(KT)
                ]

                def qsl_of(qc):
                    return slice(qc * SC, (qc + 1) * SC)

                hookn = [0]

                def hook(pair, qc, kt, attnT_tiles):
                    """PE/DMA producer work interleaved into iteration
                    (pair, qc, kt), emitted before the scores lead.  Pool
                    choice alternates globally so consecutive hook items
                    never reuse the same psum bank back to back."""
                    def next_pool():
                        pool = opps if hookn[0] % 2 == 0 else projps
                        hookn[0] += 1
                        return pool

                    for item in _HOOKS[(qc, kt)]:
                        kind = item[0]
                        if kind == "vo":
                            if pair == 0:
                                v_proj(item[1], 0, next_pool())
                        elif kind == "vp":
                            if pair < PAIRS - 1:
                                v_proj(item[1], pair + 1, next_pool())
                        elif kind == "k":
                            if pair < PAIRS - 1:
                                proj_chunk(2 * (pair + 1), item[1], next_pool())
                        elif kind == "qn":
                            if pair < PAIRS - 1:
                                proj_chunk(2 * (pair + 1) + 1, 0, next_pool())
                        elif kind == "qs":
                            # pair 3's own q chunks are prefetched from the
                            # pair-1/2 groups so its o_proj windows stay light
                            if pair < PAIRS - 1:
                                proj_chunk(2 * pair + 1, item[1], next_pool())
                    if pair == 1 and kt == 1 and qc in (1, 2):
                        proj_chunk(7, qc, next_pool())       # q pair3 c1/c2
                    if pair == 2 and kt == 1 and qc == 1:
                        proj_chunk(7, 3, next_pool())        # q pair3 c3
                    # pairs 2/3 did not get V st0-4 from the head (only the
                    # pair-1 prefetch rides there): produce them here
                    if pair in (1, 2) and qc >= 1 and kt in (8, 10):
                        st = {(1, 8): 0, (1, 10): 1, (2, 8): 2, (2, 10): 3,
                              (3, 8): 4}.get((qc, kt))
                        if st is not None:
                            v_proj(st, pair + 1, next_pool())
                    if pair == PAIRS - 1 and qc >= 1 and kt in _OPROJ_KTS:
                        m = _OPROJ_KTS.index(kt)
                        o_proj_m(qc - 1, m, attnT_tiles[qc - 1], next_pool())

                def pv_mms(pair, qc, kt, e, heads):
                    # acc banks hold four 65-col accumulation groups each; a
                    # start=True would zero the whole bank and wipe sibling
                    # groups, so the accs are memset-zeroed at kt0 and every
                    # matmul is a pure accumulate.
                    accA, accB = accs[(pair, qc)]
                    vx = vext[:, kt, :].rearrange("p (h x) -> p h x", x=65)
                    for h in heads:
                        acc = accA if h == 0 else accB
                        for qs in range(QT):
                            nc.tensor.matmul(
                                acc[:, qs, :],
                                e[:, h, qs * 128 : (qs + 1) * 128],
                                vx[:, 2 * pair + h, :],
                                start=False,
                                stop=(kt == KT - 1),
                                skip_group_check=True,
                            )

                s2q = {
                    0: emit_scores(flat[0][0], flat[0][2], qsl_of(flat[0][1])),
                    1: emit_scores(flat[1][0], flat[1][2], qsl_of(flat[1][1])),
                    2: emit_scores(flat[2][0], flat[2][2], qsl_of(flat[2][1])),
                }
                # k pair0 c1-3 after the scores prestage, via the proj pools
                # so they do not recycle the prestaged scores banks
                proj_chunk(0, 1, projps, copy_eng="scalar")
                proj_chunk(0, 2, opps, copy_eng="scalar")
                proj_chunk(0, 3, projps, copy_eng="scalar")
                accs = {}
                attn_tiles = {}
                attnT_tiles = {}
                ekt = {}
                for i, (pair, qc, kt) in enumerate(flat):
                    if pair == 0 and kt == 0:
                        attn_tiles[qc] = attns.tile(
                            [128, PAIRS, QT, 128], BF, tag="attn", name="attn"
                        )
                    if kt == 0:
                        acAt = pvps.tile([128, SC], F32, tag="acA", name="acAt")
                        acBt = pvps.tile([128, SC], F32, tag="acB", name="acBt")
                        accA = acAt[:, 0 : QT * 65].rearrange(
                            "p (q x) -> p q x", x=65
                        )
                        accB = acBt[:, 0 : QT * 65].rearrange(
                            "p (q x) -> p q x", x=65
                        )
                        accs[(pair, qc)] = (accA, accB)
                        nc.vector.memset(accA, 0.0)
                        nc.vector.memset(accB, 0.0)
                    hook(pair, qc, kt, attnT_tiles)
                    s2 = s2q.pop(i)
                    e = eps.tile([128, 2, SC], BF)
                    nc.scalar.activation(out=e[:], in_=s2[:], func=Exp, scale=EXP_SCALE)
                    if i + 3 < len(flat):
                        npair, nqc, nkt = flat[i + 3]
                        s2q[i + 3] = emit_scores(npair, nkt, qsl_of(nqc))
                    # P@V lags exp: head A by one iteration, head B by two.
                    # kt0's P@V waits on the previous window's normalize (psum
                    # WAR); the lag absorbs that latency and keeps any parked
                    # group within the 4-deep PE wait queue.
                    ekt[kt] = e
                    if kt >= 2:
                        pv_mms(pair, qc, kt - 2, ekt[kt - 2], (0,))
                    if kt >= 3:
                        pv_mms(pair, qc, kt - 3, ekt[kt - 3], (1,))
                    if kt == KT - 1:
                        pv_mms(pair, qc, KT - 2, ekt[KT - 2], (0,))
                        pv_mms(pair, qc, KT - 1, ekt[KT - 1], (0,))
                        pv_mms(pair, qc, KT - 3, ekt[KT - 3], (1,))
                        pv_mms(pair, qc, KT - 2, ekt[KT - 2], (1,))
                        pv_mms(pair, qc, KT - 1, ekt[KT - 1], (1,))
                        rec = recs.tile([128, 2, QT], F32)
                        accA, accB = accs[(pair, qc)]
                        at = attn_tiles[qc]
                        nc.vector.reciprocal(rec[:, 0, :], accA[:, :, 64])
                        nc.vector.reciprocal(rec[:, 1, :], accB[:, :, 64])
                        for h, acc in ((0, accA), (1, accB)):
                            for qs in range(QT):
                                nc.vector.tensor_scalar_mul(
                                    at[:, pair, qs, h * 64 : (h + 1) * 64],
                                    acc[:, qs, 0:64],
                                    rec[:, h, qs : qs + 1],
                                )
                        # transpose this pair's attn [q, d] -> [d, q] in ONE
                        # XBAR DMA (no PE/PSUM); attnT(qc) complete once the
                        # pair-3 transpose lands
                        if pair == 0:
                            attnT_tiles[qc] = attnTs.tile(
                                [128, PAIRS, SC], BF, tag="aT", name="aT"
                            )
                        nc.sync.dma_start_transpose(
                            out=attnT_tiles[qc][:, pair, :].rearrange(
                                "p (di m) -> p di m", m=128
                            ),
                            in_=at[:, pair, :, :],
                        )

                # ---- tail: last chunk's o_proj.  Six psum slots (opps,
                # projps, and the now-idle scores banks) let the ot 0-2
                # matmuls prefill while the pair-3 transpose is in flight;
                # the ot-3 matmuls and copies then stream without bank waits.
                attnT_q = attnT_tiles[NSC - 1]
                t_s2a = sps.tile([128, 2, SC], F32, tag="s2", name="t_s2a")
                t_s2b = sps.tile([128, 2, SC], F32, tag="s2", name="t_s2b")
                t_pva = pvps.tile([128, SC], F32, tag="acA", name="t_pva")
                t_pvb = pvps.tile([128, SC], F32, tag="acB", name="t_pvb")
                t_slots = [
                    alloc_ps(opps),
                    alloc_ps(projps),
                    t_s2a[:, 0, :],
                    t_s2a[:, 1, :],
                    t_s2b[:, 0, :],
                    t_s2b[:, 1, :],
                    t_pva[:],
                    t_pvb[:],
                ]
                qsl = qsl_of(NSC - 1)

                def t_oproj_mms(m, ots):
                    op = t_slots[m]
                    for ot in ots:
                        nc.tensor.matmul(
                            op,
                            wo_sb[:, ot, m * 128 : (m + 1) * 128],
                            attnT_q[:, ot, :],
                            start=(ot == 0),
                            stop=(ot == VF // 128 - 1),
                        )

                def t_oproj_fin(m, grp):
                    t_oproj_mms(m, (3,))
                    # ACT is idle after the last exp: share the tail copies
                    if m % 2 == 0:
                        nc.vector.tensor_copy(grp[:, m % 4, :], t_slots[m])
                    else:
                        nc.scalar.copy(grp[:, m % 4, :], t_slots[m])

                for m in range(HT):
                    t_oproj_mms(m, (0, 1, 2))
                for g in range(2):
                    grp = obg.tile([128, 4, SC], BF, tag="og", name="og")
                    for m in range(4 * g, 4 * g + 4):
                        t_oproj_fin(m, grp)
                    # one batched DMA per 4 output tiles: the tail is HWDGE
                    # setup-latency bound, not bandwidth bound
                    nc.sync.dma_start(
                        out=outT[g * SC : (g + 1) * SC, qsl].rearrange(
                            "(m p) s -> p m s", p=128
                        ),
                        in_=grp[:],
                    )
    _split_excess_waits(nc)
    return nc


def _prep_inputs(cos, sin, hidden_states, w_qkv, w_o):
    """Per-core host-side sharding/transpose/cast. Returns list of in_maps."""
    bf = np.float16
    cos = np.asarray(cos, np.float32)
    sin = np.asarray(sin, np.float32)
    hidden_states = np.asarray(hidden_states, np.float32)
    w_qkv = np.asarray(w_qkv, np.float32)
    w_o = np.asarray(w_o, np.float32)

    cosT = cos.T  # [64, S]
    cos_t = np.ascontiguousarray(np.tile(cosT, (2, 1))).astype(bf)
    # sin multiplier aligned to the *source* partitions of the rot ops:
    # rows [0:32] = +sin[32:64] (multiplies src q[0:32] -> dest [32:64]),
    # rows [32:64] = -sin[0:32] (multiplies src q[32:64] -> dest [0:32]).
    sinT = sin.T
    sin_t = np.ascontiguousarray(
        np.tile(np.concatenate([sinT[32:], -sinT[:32]], 0), (2, 1))
    ).astype(bf)

    in_maps = []
    for core in range(8):
        b, g = core // 2, core % 2
        hT = hidden_states[b].T  # [H, S]
        hT_t = np.ascontiguousarray(
            hT.reshape(HT, 128, S).transpose(1, 0, 2)
        ).astype(bf)
        qs, ks, vs = g * VF, NH * HD + g * VF, 2 * NH * HD + g * VF
        # interleave per pair: [k_p0, q_p0, k_p1, q_p1, ...]
        blocks = []
        for p in range(PAIRS):
            blocks.append(w_qkv[ks + p * 128 : ks + (p + 1) * 128])
            blocks.append(w_qkv[qs + p * 128 : qs + (p + 1) * 128])
        wqk_rows = np.concatenate(blocks, 0)  # [QKF, H]
        wqk_t = np.ascontiguousarray(
            wqk_rows.T.reshape(HT, 128, QKF).transpose(1, 0, 2)
        ).astype(bf)
        wv_t = np.ascontiguousarray(
            w_qkv[vs : vs + VF].T.reshape(HT, 128, VF).transpose(1, 0, 2)
        ).astype(bf)
        woT = w_o[:, g * VF : (g + 1) * VF].T  # [VF, H]
        wo_t = np.ascontiguousarray(
            woT.reshape(VF // 128, 128, H).transpose(1, 0, 2)
        ).astype(bf)
        in_maps.append(
            {
                "hT": hT_t,
                "wqk": wqk_t,
                "wv": wv_t,
                "wo": wo_t,
                "cos_t": cos_t,
                "sin_t": sin_t,
            }
        )
    return in_maps


def kernel(cos, sin, hidden_states, w_qkv, w_o, _trace=False):
    global _CACHED_NC
    if _CACHED_NC is None:
        _CACHED_NC = _build_nc()
    nc = _CACHED_NC
    in_maps = _prep_inputs(cos, sin, hidden_states, w_qkv, w_o)
    res = run_bass_kernel_spmd(nc, in_maps, core_ids=list(range(8)), trace=_trace)
    outs = [r["outT"] for r in res.results]
    out = np.empty((B, S, H), np.float32)
    for b in range(B):
        out[b] = (
            outs[2 * b].astype(np.float32) + outs[2 * b + 1].astype(np.float32)
        ).T
    if _trace:
        return out, res
    return out
